# revision 7
# baseline (speedup 1.0000x reference)
"""Trainium2 Bass kernel for nn_DeepConvectionNet.

Strategy (data-parallel over batch, one sample per NeuronCore, 8 cores):
  per core:
    1. compute the 5 patch fields (p_norm, r_x, r_y, u_norm, v_norm) on-chip
       from the raw sample, write them (plus raw u, v) as zero-padded
       [138, 262] planes to DRAM scratch.
    2. stream "REP" tiles from the planes: REP[k=(g,f,dx), j] =
       plane_f[j + g*262 + dx], so the L1 matmul of the 7x7x5 stencil MLP
       becomes 3 accumulating matmuls (K=105,105,35) whose rhs are strided
       slices of REP — no im2col materialisation.
    3. run the 6-layer MLP stack (tanh every layer) in "transposed"
       activation layout [features(part), cells(free)] with float32r
       matmuls (full PE rate, ~1e-4 rounding) + ScalarE tanh.
    4. heads: elementwise multiply of the 10 outputs with gathered
       velocity taps, reduced over the 5 taps by a tiny K=10 matmul.
    5. results are staged in cell order and written out as two flat rows.
"""
import sys

for _p in ("/opt/trn_rl_repo", "/root/.axon_site/_ro/trn_rl_repo"):
    if _p not in sys.path:
        sys.path.append(_p)

import numpy as np

import bass_rust
import concourse.bass as bass
import concourse.bacc as bacc
import concourse.bass_isa as bass_isa
import concourse.tile as tile
from concourse import mybir
from concourse import bass2jax

f32 = mybir.dt.float32
f32r = mybir.dt.float32r
AF = mybir.ActivationFunctionType
ALU = mybir.AluOpType
AX = mybir.AxisListType

N_CORES = 8
B, NY, NX = 8, 128, 256
N = NY * NX
PLW = NX + 6          # 262
PLROWS = 138
PLANE = PLROWS * PLW  # 36156
NBLK = 16
BR = NY // NBLK       # 8 grid rows per block
FD = BR * NX // 2     # 1024 cells per chain per block
FBLK = 3664           # REP tile free length (>= 6*262+7*262+255+1)
FH = 2096             # huv tile free length (>= 7*262+255+1)
P_ATM = 101325.0
EPS = 1e-8

# layer dims
D1, D2, D3, D4, D5, D6 = 214, 185, 156, 127, 68, 10
L1_GROUPS = [(0, 105), (3, 105), (6, 35)]   # (base_dy, K)


def _layout():
    """Column layout of the packed weight / bias images."""
    wcols = {}
    c = 0

    def add(name, k, d):
        nonlocal c
        wcols[name] = (c, k, d)
        c += d

    for ch in ("x", "y"):
        for gi, (_, k) in enumerate(L1_GROUPS):
            add(f"w1{ch}{gi}a", k, 128)
            add(f"w1{ch}{gi}b", k, D1 - 128)
        add(f"w2{ch}ka_a", 128, 128)
        add(f"w2{ch}kb_a", D1 - 128, 128)
        add(f"w2{ch}ka_b", 128, D2 - 128)
        add(f"w2{ch}kb_b", D1 - 128, D2 - 128)
        add(f"w3{ch}ka_a", 128, 128)
        add(f"w3{ch}kb_a", D2 - 128, 128)
        add(f"w3{ch}ka_b", 128, D3 - 128)
        add(f"w3{ch}kb_b", D2 - 128, D3 - 128)
        add(f"w4{ch}ka", 128, D4)
        add(f"w4{ch}kb", D3 - 128, D4)
    add("w5", D4, D5)
    add("w6", D5, D6)
    add("e0", D6, 1)
    add("e1", D6, 1)
    wc = c

    bcols = {}
    c = 0

    def addb(name, d):
        nonlocal c
        bcols[name] = (c, d)
        c += 1

    for ch in ("x", "y"):
        for nm, d in (("b1a", 128), ("b1b", D1 - 128), ("b2a", 128),
                      ("b2b", D2 - 128), ("b3a", 128), ("b3b", D3 - 128),
                      ("b4", D4), ("b5", D5), ("b6", D6)):
            addb(nm + ch, d)
    return wcols, wc, bcols, c


WCOLS, WC, BCOLS, BC = _layout()


def _permute_w1(w1):
    """[245, D1] reference feature order (dy*7+dx)*5+f -> 3 groups in
    (g, f, dx) row order."""
    groups = []
    for base_dy, ng in ((0, 3), (3, 3), (6, 1)):
        g_rows = np.empty((35 * ng, w1.shape[1]), np.float32)
        for g in range(ng):
            for fch in range(5):
                for dx in range(7):
                    g_rows[g * 35 + fch * 7 + dx] = w1[((base_dy + g) * 7 + dx) * 5 + fch]
        groups.append(g_rows)
    return groups


def _pack_params(params):
    wm = np.zeros((128, WC), np.float32)
    bm = np.zeros((128, BC), np.float32)

    def put(name, block):
        c0, k, d = WCOLS[name]
        assert block.shape == (k, d), (name, block.shape, (k, d))
        wm[:k, c0:c0 + d] = block

    def putb(name, vec):
        c0, d = BCOLS[name]
        assert vec.shape == (d,)
        bm[:d, c0] = vec

    for ch, loc in (("x", params["local_x"]), ("y", params["local_y"])):
        W = [np.asarray(w, np.float32) for w in loc["W"]]
        bb = [np.asarray(b, np.float32) for b in loc["b"]]
        g1 = _permute_w1(W[0])
        for gi in range(3):
            put(f"w1{ch}{gi}a", g1[gi][:, 0:128])
            put(f"w1{ch}{gi}b", g1[gi][:, 128:D1])
        put(f"w2{ch}ka_a", W[1][0:128, 0:128])
        put(f"w2{ch}kb_a", W[1][128:D1, 0:128])
        put(f"w2{ch}ka_b", W[1][0:128, 128:D2])
        put(f"w2{ch}kb_b", W[1][128:D1, 128:D2])
        put(f"w3{ch}ka_a", W[2][0:128, 0:128])
        put(f"w3{ch}kb_a", W[2][128:D2, 0:128])
        put(f"w3{ch}ka_b", W[2][0:128, 128:D3])
        put(f"w3{ch}kb_b", W[2][128:D2, 128:D3])
        put(f"w4{ch}ka", W[3][0:128, :])
        put(f"w4{ch}kb", W[3][128:D3, :])
        putb("b1a" + ch, bb[0][0:128])
        putb("b1b" + ch, bb[0][128:D1])
        putb("b2a" + ch, bb[1][0:128])
        putb("b2b" + ch, bb[1][128:D2])
        putb("b3a" + ch, bb[2][0:128])
        putb("b3b" + ch, bb[2][128:D3])
        putb("b4" + ch, bb[3])
    Wg = [np.asarray(w, np.float32) for w in params["global"]["W"]]
    bg = [np.asarray(b, np.float32) for b in params["global"]["b"]]
    put("w5", Wg[0])
    put("w6", Wg[1])
    for ch in ("x", "y"):
        putb("b5" + ch, bg[0])
        putb("b6" + ch, bg[1])
    e0 = np.zeros((D6, 1), np.float32)
    e0[0:5] = 1.0
    e1 = np.zeros((D6, 1), np.float32)
    e1[5:10] = 1.0
    put("e0", e0)
    put("e1", e1)
    return wm, bm


def _ap(tile_ap, extra_off, dims):
    return bass_rust.AP(tile_ap.tensor, tile_ap.offset + extra_off, dims)


def _build():
    nc = bacc.Bacc("TRN2", target_bir_lowering=False, debug=False,
                   num_devices=N_CORES)
    u_in = nc.declare_dram_parameter("u", [N], f32, isOutput=False)
    v_in = nc.declare_dram_parameter("v", [N], f32, isOutput=False)
    p_in = nc.declare_dram_parameter("p", [N], f32, isOutput=False)
    wm_in = nc.declare_dram_parameter("wmega", [128, WC], f32, isOutput=False)
    bm_in = nc.declare_dram_parameter("bmega", [128, BC], f32, isOutput=False)
    xe_out = nc.declare_dram_parameter("xedge", [N], f32, isOutput=True)
    ye_out = nc.declare_dram_parameter("yedge", [N], f32, isOutput=True)
    planes = nc.dram_tensor("planes", [7 * PLANE], f32)
    edges = nc.dram_tensor("edges", [2, 2, N // 2], f32)

    with tile.TileContext(nc) as tc:
        with tc.tile_pool(name="const", bufs=1) as cpool, \
             tc.tile_pool(name="stage", bufs=4) as spool, \
             tc.tile_pool(name="repm", bufs=2) as rpool, \
             tc.tile_pool(name="huv", bufs=3) as hpool, \
             tc.tile_pool(name="acts", bufs=2) as apool, \
             tc.tile_pool(name="prod", bufs=2) as ppool, \
             tc.tile_pool(name="psum", bufs=2, space="PSUM") as qpool:

            wm = cpool.tile([128, WC], f32r, tag="wm")
            nc.sync.dma_start(wm[:], wm_in[:].bitcast(f32r))
            bm = cpool.tile([128, BC], f32, tag="bm")
            nc.sync.dma_start(bm[:], bm_in[:])


            # ---------------- field computation ----------------
            with tc.tile_pool(name="fields", bufs=1) as fpool:
                u_t = fpool.tile([NY, NX], f32, tag="u")
                v_t = fpool.tile([NY, NX], f32, tag="v")
                p_t = fpool.tile([NY, NX], f32, tag="p")
                nc.sync.dma_start(u_t[:], u_in[:].rearrange("(a b) -> a b", b=NX))
                nc.sync.dma_start(v_t[:], v_in[:].rearrange("(a b) -> a b", b=NX))
                nc.sync.dma_start(p_t[:], p_in[:].rearrange("(a b) -> a b", b=NX))

                # padded field tiles [NY, PLW]
                def padtile(tag):
                    t = fpool.tile([NY, PLW], f32, tag=tag)
                    nc.vector.memset(t[:], 0.0)
                    return t

                pn, rx, ry, un, vn, up, vp = (padtile(t) for t in
                                              ("pn", "rx", "ry", "un", "vn", "up", "vp"))
                # p_norm
                nc.vector.tensor_scalar_add(pn[:, 3:3 + NX], p_t[:], -P_ATM)
                # raw u, v (padded)
                nc.vector.tensor_copy(up[:, 3:3 + NX], u_t[:])
                nc.vector.tensor_copy(vp[:, 3:3 + NX], v_t[:])

                # u_norm / v_norm
                for src, dst, tagp in ((u_t, un, "nu"), (v_t, vn, "nv")):
                    mx = fpool.tile([NY, 1], f32, tag=tagp + "mx")
                    nc.vector.tensor_reduce(mx[:], src[:], axis=AX.X, op=ALU.max)
                    am = fpool.tile([NY, 1], f32, tag=tagp + "am")
                    nc.gpsimd.partition_all_reduce(am[:], mx[:], NY,
                                                   bass_isa.ReduceOp.max)
                    nc.vector.tensor_scalar_add(am[:], am[:], EPS)
                    rcp = fpool.tile([NY, 1], f32, tag=tagp + "rc")
                    nc.vector.reciprocal(rcp[:], am[:])
                    nc.vector.tensor_scalar_mul(dst[:, 3:3 + NX], src[:], rcp[:, 0:1])

                # r_x from u: d = shift(-1 in x, clamp last), up = shift(+1, inlet=1)
                dsh = fpool.tile([NY, NX], f32, tag="dsh")
                nc.vector.tensor_copy(dsh[:, 0:NX - 1], u_t[:, 1:NX])
                nc.vector.tensor_copy(dsh[:, NX - 1:NX], u_t[:, NX - 1:NX])
                ush = fpool.tile([NY, NX], f32, tag="ush")
                nc.vector.memset(ush[:, 0:1], 1.0)
                nc.vector.tensor_copy(ush[:, 1:NX], u_t[:, 0:NX - 1])
                num = fpool.tile([NY, NX], f32, tag="num")
                nc.vector.tensor_tensor(num[:], u_t[:], ush[:], op=ALU.subtract)
                den = fpool.tile([NY, NX], f32, tag="den")
                nc.vector.tensor_tensor(den[:], dsh[:], u_t[:], op=ALU.subtract)
                nc.vector.tensor_scalar_add(den[:], den[:], EPS)
                rcp2 = fpool.tile([NY, NX], f32, tag="rcp2")
                nc.vector.reciprocal(rcp2[:], den[:])
                rr = fpool.tile([NY, NX], f32, tag="rr")
                nc.vector.tensor_tensor(rr[:], num[:], rcp2[:], op=ALU.mult)
                nc.vector.tensor_scalar(rx[:, 3:3 + NX], rr[:], 0.0, 2.0,
                                        op0=ALU.max, op1=ALU.min)

                # r_y from v: d = row above (zero BC), u = row below (zero BC)
                dsh2 = fpool.tile([NY, NX], f32, tag="dsh2")
                nc.vector.memset(dsh2[0:1, :], 0.0)
                nc.sync.dma_start(dsh2[1:NY, :], v_t[0:NY - 1, :])
                ush2 = fpool.tile([NY, NX], f32, tag="ush2")
                nc.vector.memset(ush2[:], 0.0)
                nc.sync.dma_start(ush2[0:NY - 1, :], v_t[1:NY, :])
                num2 = fpool.tile([NY, NX], f32, tag="num")
                nc.vector.tensor_tensor(num2[:], v_t[:], ush2[:], op=ALU.subtract)
                den2 = fpool.tile([NY, NX], f32, tag="den")
                nc.vector.tensor_tensor(den2[:], dsh2[:], v_t[:], op=ALU.subtract)
                nc.vector.tensor_scalar_add(den2[:], den2[:], EPS)
                rcp3 = fpool.tile([NY, NX], f32, tag="rcp2")
                nc.vector.reciprocal(rcp3[:], den2[:])
                rr2 = fpool.tile([NY, NX], f32, tag="rr")
                nc.vector.tensor_tensor(rr2[:], num2[:], rcp3[:], op=ALU.mult)
                nc.vector.tensor_scalar(ry[:, 3:3 + NX], rr2[:], 0.0, 2.0,
                                        op0=ALU.max, op1=ALU.min)

                # write planes (interior rows 3..130 contiguous since width==PLW)
                zt = cpool.tile([7, PLW], f32, tag="zt")
                nc.vector.memset(zt[:], 0.0)
                for fi, ft in enumerate((pn, rx, ry, un, vn, up, vp)):
                    base = fi * PLANE
                    nc.sync.dma_start(
                        _ap(planes[:], base + 3 * PLW, [[PLW, NY], [1, PLW]]), ft[:])
                    nc.sync.dma_start(
                        _ap(planes[:], base, [[PLW, 3], [1, PLW]]), zt[0:3, :])
                    nc.sync.dma_start(
                        _ap(planes[:], base + 131 * PLW, [[PLW, 7], [1, PLW]]), zt[:])

            # ---------------- main loop ----------------
            for blk in range(NBLK):
                j0 = blk * BR * PLW
                repm = rpool.tile([105, FBLK], f32r, tag="repm")
                rap = repm[:]
                rpitch = rap.ap[0][0]
                for g in range(3):
                    for fch in range(5):
                        r0 = g * 35 + fch * 7
                        nc.sync.dma_start(
                            repm[r0:r0 + 7, :],
                            _ap(planes[:], fch * PLANE + j0 + g * PLW,
                                [[1, 7], [1, FBLK]]).bitcast(f32r))
                huv = {}
                for ch, xpar in (("x", 0), ("y", 1)):
                    t = hpool.tile([10, FH], f32, tag="huv")
                    if ch == "x":
                        off_u = 5 * PLANE + j0 + 3 * PLW + 1
                        off_v = 6 * PLANE + j0 + 3 * PLW + 1
                        dims = [[1, 5], [1, FH]]
                    else:
                        off_u = 5 * PLANE + j0 + PLW + 3
                        off_v = 6 * PLANE + j0 + PLW + 3
                        dims = [[PLW, 5], [1, FH]]
                    nc.sync.dma_start(t[0:5, :], _ap(planes[:], off_u, dims))
                    nc.sync.dma_start(t[5:10, :], _ap(planes[:], off_v, dims))
                    huv[ch] = t

                for ch, xpar in (("x", 0), ("y", 1)):
                    def wslice(name):
                        c0, k, d = WCOLS[name]
                        return wm[0:k, c0:c0 + d]

                    def bslice(name, d0, dn):
                        c0, _ = BCOLS[name + ch]
                        return bm[d0:dn, c0:c0 + 1]

                    ns = [slice(0, 512), slice(512, 1024)]

                    # L1
                    psA = qpool.tile([128, FD], f32, tag="psA")
                    psB = qpool.tile([128, FD], f32, tag="psB")
                    for n in range(2):
                        for gi, (bdy, K) in enumerate(L1_GROUPS):
                            rhs = _ap(rap, bdy * PLW + n * 4 * PLW + xpar,
                                      [[rpitch, K], [PLW, 4], [2, 128]])
                            nc.tensor.matmul(psA[:, ns[n]], wslice(f"w1{ch}{gi}a"),
                                             rhs, start=(gi == 0), stop=(gi == 2))
                        for gi, (bdy, K) in enumerate(L1_GROUPS):
                            rhs = _ap(rap, bdy * PLW + n * 4 * PLW + xpar,
                                      [[rpitch, K], [PLW, 4], [2, 128]])
                            nc.tensor.matmul(psB[0:D1 - 128, ns[n]],
                                             wslice(f"w1{ch}{gi}b"),
                                             rhs, start=(gi == 0), stop=(gi == 2))
                    t1a = apool.tile([128, FD], f32r, tag="t1a")
                    t1b = apool.tile([D1 - 128, FD], f32r, tag="t1b")
                    nc.scalar.activation(t1a[:], psA[:], AF.Tanh,
                                         bias=bslice("b1a", 0, 128))
                    nc.scalar.activation(t1b[:], psB[0:D1 - 128, :], AF.Tanh,
                                         bias=bslice("b1b", 0, D1 - 128))

                    # L2 / L3 pattern
                    def layer2(nm, ka_t, kb_t, kb_n, douts, btags, outtags):
                        outs = []
                        for which, (d0, dn) in enumerate(douts):
                            ps = qpool.tile([128, FD], f32,
                                            tag=("psA" if which == 0 else "psB"))
                            dd = dn - d0
                            for n in range(2):
                                nc.tensor.matmul(ps[0:dd, ns[n]],
                                                 wslice(f"{nm}{ch}ka_{'ab'[which]}"),
                                                 ka_t[:, ns[n]], start=True, stop=False)
                                nc.tensor.matmul(ps[0:dd, ns[n]],
                                                 wslice(f"{nm}{ch}kb_{'ab'[which]}"),
                                                 kb_t[0:kb_n, ns[n]], start=False,
                                                 stop=True)
                            ot = apool.tile([dd, FD], f32r, tag=outtags[which])
                            nc.scalar.activation(ot[:], ps[0:dd, :], AF.Tanh,
                                                 bias=bslice(btags[which], 0, dd))
                            outs.append(ot)
                        return outs

                    t2a, t2b = layer2("w2", t1a, t1b, D1 - 128,
                                      [(0, 128), (128, D2)], ("b2a", "b2b"),
                                      ("t2a", "t2b"))
                    t3a, t3b = layer2("w3", t2a, t2b, D2 - 128,
                                      [(0, 128), (128, D3)], ("b3a", "b3b"),
                                      ("t3a", "t3b"))

                    # L4: douts D4=127, K = 128 + 28
                    ps4 = qpool.tile([128, FD], f32, tag="psA")
                    for n in range(2):
                        nc.tensor.matmul(ps4[0:D4, ns[n]], wslice(f"w4{ch}ka"),
                                         t3a[:, ns[n]], start=True, stop=False)
                        nc.tensor.matmul(ps4[0:D4, ns[n]], wslice(f"w4{ch}kb"),
                                         t3b[:, ns[n]], start=False, stop=True)
                    t4 = apool.tile([D4, FD], f32r, tag="t4")
                    nc.scalar.activation(t4[:], ps4[0:D4, :], AF.Tanh,
                                         bias=bslice("b4", 0, D4))

                    # L5: K=127 -> 68
                    ps5 = qpool.tile([128, FD], f32, tag="psB")
                    for n in range(2):
                        nc.tensor.matmul(ps5[0:D5, ns[n]], wslice("w5"),
                                         t4[:, ns[n]], start=True, stop=True)
                    t5 = apool.tile([D5, FD], f32r, tag="t5")
                    nc.scalar.activation(t5[:], ps5[0:D5, :], AF.Tanh,
                                         bias=bslice("b5", 0, D5))

                    # L6: K=68 -> 10
                    ps6 = qpool.tile([128, FD], f32, tag="psA")
                    for n in range(2):
                        nc.tensor.matmul(ps6[0:D6, ns[n]], wslice("w6"),
                                         t5[:, ns[n]], start=True, stop=True)
                    t6 = apool.tile([D6, FD], f32r, tag="t6")
                    nc.scalar.activation(t6[:], ps6[0:D6, :], AF.Tanh,
                                         bias=bslice("b6", 0, D6))

                    # heads
                    prod = ppool.tile([D6, FD], f32r, tag="prod")
                    hap = huv[ch][:]
                    hpitch = hap.ap[0][0]
                    hin = _ap(hap, xpar, [[hpitch, D6], [PLW, BR], [2, 128]])
                    nc.vector.tensor_tensor(prod[:], t6[:].bitcast(f32),
                                            hin, op=ALU.mult)
                    psh0 = qpool.tile([1, FD], f32, tag="psB")
                    psh1 = qpool.tile([1, FD], f32, tag="psB")
                    for n in range(2):
                        nc.tensor.matmul(psh0[0:1, ns[n]], wslice("e0"),
                                         prod[:, ns[n]], start=True, stop=True)
                        nc.tensor.matmul(psh1[0:1, ns[n]], wslice("e1"),
                                         prod[:, ns[n]], start=True, stop=True)
                    stx = spool.tile([1, FD], f32, tag="st")
                    sty = spool.tile([1, FD], f32, tag="st")
                    nc.vector.tensor_copy(stx[:], psh0[0:1, :])
                    nc.vector.tensor_copy(sty[:], psh1[0:1, :])
                    nc.sync.dma_start(edges[0, xpar, blk * FD:(blk + 1) * FD], stx[:])
                    nc.sync.dma_start(edges[1, xpar, blk * FD:(blk + 1) * FD], sty[:])

            # epilogue: interleave even/odd faces back into cell order
            with tc.tile_pool(name="epi", bufs=1) as epool:
                for oi, out_t in ((0, xe_out), (1, ye_out)):
                    ot = epool.tile([NY, NX], f32, tag=f"oint{oi}")
                    opitch = ot[:].ap[0][0]
                    for xpar in (0, 1):
                        half = epool.tile([NY, NX // 2], f32, tag=f"oh{oi}{xpar}")
                        nc.sync.dma_start(
                            half[:],
                            edges[oi, xpar, :].rearrange("(a b) -> a b", b=NX // 2))
                        nc.vector.tensor_copy(
                            _ap(ot[:], xpar, [[opitch, NY], [2, NX // 2]]), half[:])
                    nc.sync.dma_start(out_t[:].rearrange("(a b) -> a b", b=NX), ot[:])
    nc.compile()
    return nc


_CACHE = {}


def _get_runner():
    if "runner" in _CACHE:
        return _CACHE["runner"]

    import jax
    from jax.experimental.shard_map import shard_map
    from jax.sharding import Mesh, PartitionSpec

    nc = _build()
    bass2jax.install_neuronx_cc_hook()

    partition_name = (nc.partition_id_tensor.name
                      if nc.partition_id_tensor is not None else None)
    in_names = []
    out_names = []
    out_avals = []
    for alloc in nc.m.functions[0].allocations:
        if not isinstance(alloc, mybir.MemoryLocationSet):
            continue
        name = alloc.memorylocations[0].name
        if alloc.kind == "ExternalInput":
            if name != partition_name:
                in_names.append(name)
        elif alloc.kind == "ExternalOutput":
            out_names.append(name)
            out_avals.append(jax.core.ShapedArray(tuple(alloc.tensor_shape),
                                                  mybir.dt.np(alloc.dtype)))
    n_params = len(in_names)
    n_outs = len(out_names)
    all_in_names = in_names + out_names
    if partition_name is not None:
        all_in_names = all_in_names + [partition_name]

    def _body(*args):
        operands = list(args)
        if partition_name is not None:
            operands.append(bass2jax.partition_id_tensor())
        outs = bass2jax._bass_exec_p.bind(
            *operands,
            out_avals=tuple(out_avals),
            in_names=tuple(all_in_names),
            out_names=tuple(out_names),
            lowering_input_output_aliases=(),
            sim_require_finite=True,
            sim_require_nnan=True,
            nc=nc,
        )
        return tuple(outs)

    devices = jax.devices()[:N_CORES]
    mesh = Mesh(np.asarray(devices), ("core",))
    in_specs = (PartitionSpec("core"),) * (n_params + n_outs)
    out_specs = (PartitionSpec("core"),) * n_outs
    donate = tuple(range(n_params, n_params + n_outs))
    sharded = jax.jit(
        shard_map(_body, mesh=mesh, in_specs=in_specs, out_specs=out_specs,
                  check_rep=False),
        donate_argnums=donate, keep_unused=True)

    zero_out_shapes = [(N_CORES * a.shape[0], *a.shape[1:]) for a in out_avals]
    zero_out_dtypes = [a.dtype for a in out_avals]

    def run(in_maps):
        concat_in = [
            np.concatenate([np.asarray(m[name]) for m in in_maps], axis=0)
            for name in in_names
        ]
        zeros = [np.zeros(s, d) for s, d in zip(zero_out_shapes, zero_out_dtypes)]
        out_arrs = sharded(*concat_in, *zeros)
        return [
            {name: np.asarray(out_arrs[i]).reshape(N_CORES, *out_avals[i].shape)[c]
             for i, name in enumerate(out_names)}
            for c in range(N_CORES)
        ]

    _CACHE["runner"] = run
    return run


def kernel(x_velocity, y_velocity, p, x_faces, y_faces, params):
    x_velocity = np.asarray(x_velocity, np.float32)
    y_velocity = np.asarray(y_velocity, np.float32)
    p = np.asarray(p, np.float32)
    assert np.array_equal(np.asarray(x_faces), np.arange(0, N, 2, np.int32))
    assert np.array_equal(np.asarray(y_faces), np.arange(1, N, 2, np.int32))

    wm, bm = _pack_params(params)
    run = _get_runner()
    in_maps = [
        {"u": x_velocity[c], "v": y_velocity[c], "p": p[c],
         "wmega": wm, "bmega": bm}
        for c in range(N_CORES)
    ]
    res = run(in_maps)
    out_x = np.stack([res[c]["xedge"] for c in range(N_CORES)])
    out_y = np.stack([res[c]["yedge"] for c in range(N_CORES)])
    return (out_x, out_y)


# revision 9
# speedup vs baseline: 1.3691x; 1.3691x over previous
"""Trainium2 Bass kernel for nn_DeepConvectionNet.

Strategy (data-parallel over batch, one sample per NeuronCore, 8 cores):
  per core:
    1. compute the 5 patch fields (p_norm, r_x, r_y, u_norm, v_norm) on-chip
       from the raw sample, write them (plus raw u, v) as zero-padded
       [138, 262] planes to DRAM scratch.
    2. stream "REP" tiles from the planes: REP[k=(g,f,dx), j] =
       plane_f[j + g*262 + dx], so the L1 matmul of the 7x7x5 stencil MLP
       becomes 3 accumulating matmuls (K=105,105,35) whose rhs are strided
       slices of REP — no im2col materialisation.
    3. run the 6-layer MLP stack (tanh every layer) in "transposed"
       activation layout [features(part), cells(free)] with float32r
       matmuls (full PE rate, ~1e-4 rounding) + ScalarE tanh.
    4. heads: elementwise multiply of the 10 outputs with gathered
       velocity taps, reduced over the 5 taps by a tiny K=10 matmul.
    5. results are staged in cell order and written out as two flat rows.
"""
import sys

for _p in ("/opt/trn_rl_repo", "/root/.axon_site/_ro/trn_rl_repo"):
    if _p not in sys.path:
        sys.path.append(_p)

import numpy as np

import bass_rust
import concourse.bass as bass
import concourse.bacc as bacc
import concourse.bass_isa as bass_isa
import concourse.tile as tile
from concourse import mybir
from concourse import bass2jax

f32 = mybir.dt.float32
f32r = mybir.dt.float32r
bf16 = mybir.dt.bfloat16
MMDT = bf16           # matmul operand dtype
AF = mybir.ActivationFunctionType
ALU = mybir.AluOpType
AX = mybir.AxisListType

N_CORES = 8
B, NY, NX = 8, 128, 256
N = NY * NX
PLW = NX + 6          # 262
PLROWS = 138
PLANE = PLROWS * PLW  # 36156
NBLK = 16
BR = NY // NBLK       # 8 grid rows per block
FD = BR * NX // 2     # 1024 cells per chain per block
FBLK = 3664           # REP tile free length (>= 6*262+7*262+255+1)
FH = 2096             # huv tile free length (>= 7*262+255+1)
FREP = 35108          # REPD row length (134*262)
FH2 = 33536           # HUVD row length
P_ATM = 101325.0
EPS = 1e-8

# layer dims
D1, D2, D3, D4, D5, D6 = 214, 185, 156, 127, 68, 10
L1_GROUPS = [(0, 105), (3, 105), (6, 35)]   # (base_dy, K)


def _layout():
    """Column layout of the packed weight / bias images."""
    wcols = {}
    c = 0

    def add(name, k, d):
        nonlocal c
        wcols[name] = (c, k, d)
        c += d

    for ch in ("x", "y"):
        for gi, (_, k) in enumerate(L1_GROUPS):
            add(f"w1{ch}{gi}a", k, 128)
            add(f"w1{ch}{gi}b", k, D1 - 128)
        add(f"w2{ch}ka_a", 128, 128)
        add(f"w2{ch}kb_a", D1 - 128, 128)
        add(f"w2{ch}ka_b", 128, D2 - 128)
        add(f"w2{ch}kb_b", D1 - 128, D2 - 128)
        add(f"w3{ch}ka_a", 128, 128)
        add(f"w3{ch}kb_a", D2 - 128, 128)
        add(f"w3{ch}ka_b", 128, D3 - 128)
        add(f"w3{ch}kb_b", D2 - 128, D3 - 128)
        add(f"w4{ch}ka", 128, D4)
        add(f"w4{ch}kb", D3 - 128, D4)
    add("w5", D4, D5)
    add("w6", D5, D6)
    add("e0", D6, 1)
    add("e1", D6, 1)
    wc = c

    bcols = {}
    c = 0

    def addb(name, d):
        nonlocal c
        bcols[name] = (c, d)
        c += 1

    for ch in ("x", "y"):
        for nm, d in (("b1a", 128), ("b1b", D1 - 128), ("b2a", 128),
                      ("b2b", D2 - 128), ("b3a", 128), ("b3b", D3 - 128),
                      ("b4", D4), ("b5", D5), ("b6", D6)):
            addb(nm + ch, d)
    return wcols, wc, bcols, c


WCOLS, WC, BCOLS, BC = _layout()


def _permute_w1(w1):
    """[245, D1] reference feature order (dy*7+dx)*5+f -> 3 groups in
    (g, f, dx) row order."""
    groups = []
    for base_dy, ng in ((0, 3), (3, 3), (6, 1)):
        g_rows = np.empty((35 * ng, w1.shape[1]), np.float32)
        for g in range(ng):
            for fch in range(5):
                for dx in range(7):
                    g_rows[g * 35 + fch * 7 + dx] = w1[((base_dy + g) * 7 + dx) * 5 + fch]
        groups.append(g_rows)
    return groups


def _pack_params(params):
    wm = np.zeros((128, WC), np.float32)
    bm = np.zeros((128, BC), np.float32)

    def put(name, block):
        c0, k, d = WCOLS[name]
        assert block.shape == (k, d), (name, block.shape, (k, d))
        wm[:k, c0:c0 + d] = block

    def putb(name, vec):
        c0, d = BCOLS[name]
        assert vec.shape == (d,)
        bm[:d, c0] = vec

    for ch, loc in (("x", params["local_x"]), ("y", params["local_y"])):
        W = [np.asarray(w, np.float32) for w in loc["W"]]
        bb = [np.asarray(b, np.float32) for b in loc["b"]]
        g1 = _permute_w1(W[0])
        for gi in range(3):
            put(f"w1{ch}{gi}a", g1[gi][:, 0:128])
            put(f"w1{ch}{gi}b", g1[gi][:, 128:D1])
        put(f"w2{ch}ka_a", W[1][0:128, 0:128])
        put(f"w2{ch}kb_a", W[1][128:D1, 0:128])
        put(f"w2{ch}ka_b", W[1][0:128, 128:D2])
        put(f"w2{ch}kb_b", W[1][128:D1, 128:D2])
        put(f"w3{ch}ka_a", W[2][0:128, 0:128])
        put(f"w3{ch}kb_a", W[2][128:D2, 0:128])
        put(f"w3{ch}ka_b", W[2][0:128, 128:D3])
        put(f"w3{ch}kb_b", W[2][128:D2, 128:D3])
        put(f"w4{ch}ka", W[3][0:128, :])
        put(f"w4{ch}kb", W[3][128:D3, :])
        putb("b1a" + ch, bb[0][0:128])
        putb("b1b" + ch, bb[0][128:D1])
        putb("b2a" + ch, bb[1][0:128])
        putb("b2b" + ch, bb[1][128:D2])
        putb("b3a" + ch, bb[2][0:128])
        putb("b3b" + ch, bb[2][128:D3])
        putb("b4" + ch, bb[3])
    Wg = [np.asarray(w, np.float32) for w in params["global"]["W"]]
    bg = [np.asarray(b, np.float32) for b in params["global"]["b"]]
    put("w5", Wg[0])
    put("w6", Wg[1])
    for ch in ("x", "y"):
        putb("b5" + ch, bg[0])
        putb("b6" + ch, bg[1])
    e0 = np.zeros((D6, 1), np.float32)
    e0[0:5] = 1.0
    e1 = np.zeros((D6, 1), np.float32)
    e1[5:10] = 1.0
    put("e0", e0)
    put("e1", e1)
    return wm, bm


def _ap(tile_ap, extra_off, dims):
    return bass_rust.AP(tile_ap.tensor, tile_ap.offset + extra_off, dims)


def _build():
    nc = bacc.Bacc("TRN2", target_bir_lowering=False, debug=False,
                   num_devices=N_CORES)
    u_in = nc.declare_dram_parameter("u", [N], f32, isOutput=False)
    v_in = nc.declare_dram_parameter("v", [N], f32, isOutput=False)
    p_in = nc.declare_dram_parameter("p", [N], f32, isOutput=False)
    wm_in = nc.declare_dram_parameter("wmega", [128, WC], f32, isOutput=False)
    bm_in = nc.declare_dram_parameter("bmega", [128, BC], f32, isOutput=False)
    xe_out = nc.declare_dram_parameter("xedge", [N], f32, isOutput=True)
    ye_out = nc.declare_dram_parameter("yedge", [N], f32, isOutput=True)
    planes = nc.dram_tensor("planes", [7 * PLANE], f32)
    repd = nc.dram_tensor("repd", [105 * FREP], MMDT)
    huvd = nc.dram_tensor("huvd", [20 * FH2], f32)
    edges = nc.dram_tensor("edges", [2, 2, N // 2], f32)

    with tile.TileContext(nc) as tc:
        with tc.tile_pool(name="const", bufs=1) as cpool, \
             tc.tile_pool(name="stage", bufs=4) as spool, \
             tc.tile_pool(name="repm", bufs=3) as rpool, \
             tc.tile_pool(name="huv", bufs=3) as hpool, \
             tc.tile_pool(name="acts", bufs=2) as apool, \
             tc.tile_pool(name="prod", bufs=2) as ppool, \
             tc.tile_pool(name="psum", bufs=2, space="PSUM") as qpool:

            wm = cpool.tile([128, WC], MMDT, tag="wm")
            nc.gpsimd.dma_start(wm[:], wm_in[:])
            bm = cpool.tile([128, BC], f32, tag="bm")
            nc.sync.dma_start(bm[:], bm_in[:])


            # ---------------- field computation ----------------
            with tc.tile_pool(name="fields", bufs=1) as fpool:
                u_t = fpool.tile([NY, NX], f32, tag="u")
                v_t = fpool.tile([NY, NX], f32, tag="v")
                p_t = fpool.tile([NY, NX], f32, tag="p")
                nc.sync.dma_start(u_t[:], u_in[:].rearrange("(a b) -> a b", b=NX))
                nc.sync.dma_start(v_t[:], v_in[:].rearrange("(a b) -> a b", b=NX))
                nc.sync.dma_start(p_t[:], p_in[:].rearrange("(a b) -> a b", b=NX))

                # padded field tiles [NY, PLW]
                def padtile(tag):
                    t = fpool.tile([NY, PLW], f32, tag=tag)
                    nc.vector.memset(t[:], 0.0)
                    return t

                pn, rx, ry, un, vn, up, vp = (padtile(t) for t in
                                              ("pn", "rx", "ry", "un", "vn", "up", "vp"))
                # p_norm
                nc.vector.tensor_scalar_add(pn[:, 3:3 + NX], p_t[:], -P_ATM)
                # raw u, v (padded)
                nc.vector.tensor_copy(up[:, 3:3 + NX], u_t[:])
                nc.vector.tensor_copy(vp[:, 3:3 + NX], v_t[:])

                # u_norm / v_norm
                for src, dst, tagp in ((u_t, un, "nu"), (v_t, vn, "nv")):
                    mx = fpool.tile([NY, 1], f32, tag=tagp + "mx")
                    nc.vector.tensor_reduce(mx[:], src[:], axis=AX.X, op=ALU.max)
                    am = fpool.tile([NY, 1], f32, tag=tagp + "am")
                    nc.gpsimd.partition_all_reduce(am[:], mx[:], NY,
                                                   bass_isa.ReduceOp.max)
                    nc.vector.tensor_scalar_add(am[:], am[:], EPS)
                    rcp = fpool.tile([NY, 1], f32, tag=tagp + "rc")
                    nc.vector.reciprocal(rcp[:], am[:])
                    nc.vector.tensor_scalar_mul(dst[:, 3:3 + NX], src[:], rcp[:, 0:1])

                # r_x from u: d = shift(-1 in x, clamp last), up = shift(+1, inlet=1)
                dsh = fpool.tile([NY, NX], f32, tag="dsh")
                nc.vector.tensor_copy(dsh[:, 0:NX - 1], u_t[:, 1:NX])
                nc.vector.tensor_copy(dsh[:, NX - 1:NX], u_t[:, NX - 1:NX])
                ush = fpool.tile([NY, NX], f32, tag="ush")
                nc.vector.memset(ush[:, 0:1], 1.0)
                nc.vector.tensor_copy(ush[:, 1:NX], u_t[:, 0:NX - 1])
                num = fpool.tile([NY, NX], f32, tag="num")
                nc.vector.tensor_tensor(num[:], u_t[:], ush[:], op=ALU.subtract)
                den = fpool.tile([NY, NX], f32, tag="den")
                nc.vector.tensor_tensor(den[:], dsh[:], u_t[:], op=ALU.subtract)
                nc.vector.tensor_scalar_add(den[:], den[:], EPS)
                rcp2 = fpool.tile([NY, NX], f32, tag="rcp2")
                nc.vector.reciprocal(rcp2[:], den[:])
                rr = fpool.tile([NY, NX], f32, tag="rr")
                nc.vector.tensor_tensor(rr[:], num[:], rcp2[:], op=ALU.mult)
                nc.vector.tensor_scalar(rx[:, 3:3 + NX], rr[:], 0.0, 2.0,
                                        op0=ALU.max, op1=ALU.min)

                # r_y from v: d = row above (zero BC), u = row below (zero BC)
                dsh2 = fpool.tile([NY, NX], f32, tag="dsh2")
                nc.vector.memset(dsh2[0:1, :], 0.0)
                nc.sync.dma_start(dsh2[1:NY, :], v_t[0:NY - 1, :])
                ush2 = fpool.tile([NY, NX], f32, tag="ush2")
                nc.vector.memset(ush2[:], 0.0)
                nc.sync.dma_start(ush2[0:NY - 1, :], v_t[1:NY, :])
                num2 = fpool.tile([NY, NX], f32, tag="num")
                nc.vector.tensor_tensor(num2[:], v_t[:], ush2[:], op=ALU.subtract)
                den2 = fpool.tile([NY, NX], f32, tag="den")
                nc.vector.tensor_tensor(den2[:], dsh2[:], v_t[:], op=ALU.subtract)
                nc.vector.tensor_scalar_add(den2[:], den2[:], EPS)
                rcp3 = fpool.tile([NY, NX], f32, tag="rcp2")
                nc.vector.reciprocal(rcp3[:], den2[:])
                rr2 = fpool.tile([NY, NX], f32, tag="rr")
                nc.vector.tensor_tensor(rr2[:], num2[:], rcp3[:], op=ALU.mult)
                nc.vector.tensor_scalar(ry[:, 3:3 + NX], rr2[:], 0.0, 2.0,
                                        op0=ALU.max, op1=ALU.min)

                # write planes (interior rows 3..130 contiguous since width==PLW)
                zt = cpool.tile([7, PLW], f32, tag="zt")
                nc.vector.memset(zt[:], 0.0)
                for fi, ft in enumerate((pn, rx, ry, un, vn, up, vp)):
                    base = fi * PLANE
                    nc.sync.dma_start(
                        _ap(planes[:], base + 3 * PLW, [[PLW, NY], [1, PLW]]), ft[:])
                    nc.sync.dma_start(
                        _ap(planes[:], base, [[PLW, 3], [1, PLW]]), zt[0:3, :])
                    nc.sync.dma_start(
                        _ap(planes[:], base + 131 * PLW, [[PLW, 7], [1, PLW]]), zt[:])

            # ---------------- REPD / HUVD build ----------------
            # REPD[k=(g,f,dx), j] = plane_f[j + g*262 + dx], cast to bf16
            JC = FREP // 4
            for g in range(3):
                for jc in range(4):
                    nc.gpsimd.dma_start(
                        _ap(repd[:], g * 35 * FREP + jc * JC,
                            [[FREP * 7, 5], [FREP, 7], [1, JC]]),
                        _ap(planes[:], g * PLW + jc * JC,
                            [[PLANE, 5], [1, 7], [1, JC]]))
            # HUVD rows: 0-4 xu, 5-9 xv, 10-14 yu, 15-19 yv
            for ri, (pl, off, step) in enumerate((
                    (5, 3 * PLW + 1, 1), (6, 3 * PLW + 1, 1),
                    (5, PLW + 3, PLW), (6, PLW + 3, PLW))):
                nc.sync.dma_start(
                    _ap(huvd[:], ri * 5 * FH2, [[FH2, 5], [1, FH2]]),
                    _ap(planes[:], pl * PLANE + off, [[step, 5], [1, FH2]]))

            # ---------------- main loop ----------------
            for blk in range(NBLK):
                j0 = blk * BR * PLW
                repm = rpool.tile([105, FBLK], MMDT, tag="repm")
                rap = repm[:]
                rpitch = rap.ap[0][0]
                nc.sync.dma_start(
                    repm[:], _ap(repd[:], j0, [[FREP, 105], [1, FBLK]]))
                huv = {}
                for ci, ch in enumerate(("x", "y")):
                    t = hpool.tile([10, FH], f32, tag="huv")
                    nc.sync.dma_start(
                        t[:], _ap(huvd[:], ci * 10 * FH2 + j0,
                                  [[FH2, 10], [1, FH]]))
                    huv[ch] = t

                for ch, xpar in (("x", 0), ("y", 1)):
                    def wslice(name):
                        c0, k, d = WCOLS[name]
                        return wm[0:k, c0:c0 + d]

                    def bslice(name, d0, dn):
                        c0, _ = BCOLS[name + ch]
                        return bm[d0:dn, c0:c0 + 1]

                    ns = [slice(0, 512), slice(512, 1024)]

                    # L1
                    psA = qpool.tile([128, FD], f32, tag="psA")
                    psB = qpool.tile([128, FD], f32, tag="psB")
                    for n in range(2):
                        for gi, (bdy, K) in enumerate(L1_GROUPS):
                            rhs = _ap(rap, bdy * PLW + n * 4 * PLW + xpar,
                                      [[rpitch, K], [PLW, 4], [2, 128]])
                            nc.tensor.matmul(psA[:, ns[n]], wslice(f"w1{ch}{gi}a"),
                                             rhs, start=(gi == 0), stop=(gi == 2))
                        for gi, (bdy, K) in enumerate(L1_GROUPS):
                            rhs = _ap(rap, bdy * PLW + n * 4 * PLW + xpar,
                                      [[rpitch, K], [PLW, 4], [2, 128]])
                            nc.tensor.matmul(psB[0:D1 - 128, ns[n]],
                                             wslice(f"w1{ch}{gi}b"),
                                             rhs, start=(gi == 0), stop=(gi == 2))
                    t1a = apool.tile([128, FD], MMDT, tag="t1a")
                    t1b = apool.tile([D1 - 128, FD], MMDT, tag="t1b")
                    nc.scalar.activation(t1a[:], psA[:], AF.Tanh,
                                         bias=bslice("b1a", 0, 128))
                    nc.scalar.activation(t1b[:], psB[0:D1 - 128, :], AF.Tanh,
                                         bias=bslice("b1b", 0, D1 - 128))

                    # L2 / L3 pattern
                    def layer2(nm, ka_t, kb_t, kb_n, douts, btags, outtags):
                        outs = []
                        for which, (d0, dn) in enumerate(douts):
                            ps = qpool.tile([128, FD], f32,
                                            tag=("psA" if which == 0 else "psB"))
                            dd = dn - d0
                            for n in range(2):
                                nc.tensor.matmul(ps[0:dd, ns[n]],
                                                 wslice(f"{nm}{ch}ka_{'ab'[which]}"),
                                                 ka_t[:, ns[n]], start=True, stop=False)
                                nc.tensor.matmul(ps[0:dd, ns[n]],
                                                 wslice(f"{nm}{ch}kb_{'ab'[which]}"),
                                                 kb_t[0:kb_n, ns[n]], start=False,
                                                 stop=True)
                            ot = apool.tile([dd, FD], MMDT, tag=outtags[which])
                            nc.scalar.activation(ot[:], ps[0:dd, :], AF.Tanh,
                                                 bias=bslice(btags[which], 0, dd))
                            outs.append(ot)
                        return outs

                    t2a, t2b = layer2("w2", t1a, t1b, D1 - 128,
                                      [(0, 128), (128, D2)], ("b2a", "b2b"),
                                      ("t2a", "t2b"))
                    t3a, t3b = layer2("w3", t2a, t2b, D2 - 128,
                                      [(0, 128), (128, D3)], ("b3a", "b3b"),
                                      ("t3a", "t3b"))

                    # L4: douts D4=127, K = 128 + 28
                    ps4 = qpool.tile([128, FD], f32, tag="psA")
                    for n in range(2):
                        nc.tensor.matmul(ps4[0:D4, ns[n]], wslice(f"w4{ch}ka"),
                                         t3a[:, ns[n]], start=True, stop=False)
                        nc.tensor.matmul(ps4[0:D4, ns[n]], wslice(f"w4{ch}kb"),
                                         t3b[:, ns[n]], start=False, stop=True)
                    t4 = apool.tile([D4, FD], MMDT, tag="t4")
                    nc.scalar.activation(t4[:], ps4[0:D4, :], AF.Tanh,
                                         bias=bslice("b4", 0, D4))

                    # L5: K=127 -> 68
                    ps5 = qpool.tile([128, FD], f32, tag="psB")
                    for n in range(2):
                        nc.tensor.matmul(ps5[0:D5, ns[n]], wslice("w5"),
                                         t4[:, ns[n]], start=True, stop=True)
                    t5 = apool.tile([D5, FD], MMDT, tag="t5")
                    nc.scalar.activation(t5[:], ps5[0:D5, :], AF.Tanh,
                                         bias=bslice("b5", 0, D5))

                    # L6: K=68 -> 10
                    ps6 = qpool.tile([128, FD], f32, tag="psA")
                    for n in range(2):
                        nc.tensor.matmul(ps6[0:D6, ns[n]], wslice("w6"),
                                         t5[:, ns[n]], start=True, stop=True)
                    t6 = apool.tile([D6, FD], MMDT, tag="t6")
                    nc.scalar.activation(t6[:], ps6[0:D6, :], AF.Tanh,
                                         bias=bslice("b6", 0, D6))

                    # heads
                    prod = ppool.tile([D6, FD], MMDT, tag="prod")
                    hap = huv[ch][:]
                    hpitch = hap.ap[0][0]
                    hin = _ap(hap, xpar, [[hpitch, D6], [PLW, BR], [2, 128]])
                    nc.vector.tensor_tensor(prod[:], t6[:],
                                            hin, op=ALU.mult)
                    psh0 = qpool.tile([1, FD], f32, tag="psB")
                    psh1 = qpool.tile([1, FD], f32, tag="psB")
                    for n in range(2):
                        nc.tensor.matmul(psh0[0:1, ns[n]], wslice("e0"),
                                         prod[:, ns[n]], start=True, stop=True)
                        nc.tensor.matmul(psh1[0:1, ns[n]], wslice("e1"),
                                         prod[:, ns[n]], start=True, stop=True)
                    stx = spool.tile([1, FD], f32, tag="st")
                    sty = spool.tile([1, FD], f32, tag="st")
                    nc.vector.tensor_copy(stx[:], psh0[0:1, :])
                    nc.vector.tensor_copy(sty[:], psh1[0:1, :])
                    nc.sync.dma_start(edges[0, xpar, blk * FD:(blk + 1) * FD], stx[:])
                    nc.sync.dma_start(edges[1, xpar, blk * FD:(blk + 1) * FD], sty[:])

            # epilogue: interleave even/odd faces back into cell order
            with tc.tile_pool(name="epi", bufs=1) as epool:
                for oi, out_t in ((0, xe_out), (1, ye_out)):
                    ot = epool.tile([NY, NX], f32, tag=f"oint{oi}")
                    opitch = ot[:].ap[0][0]
                    for xpar in (0, 1):
                        half = epool.tile([NY, NX // 2], f32, tag=f"oh{oi}{xpar}")
                        nc.sync.dma_start(
                            half[:],
                            edges[oi, xpar, :].rearrange("(a b) -> a b", b=NX // 2))
                        nc.vector.tensor_copy(
                            _ap(ot[:], xpar, [[opitch, NY], [2, NX // 2]]), half[:])
                    nc.sync.dma_start(out_t[:].rearrange("(a b) -> a b", b=NX), ot[:])
    nc.compile()
    return nc


_CACHE = {}


def _get_runner():
    if "runner" in _CACHE:
        return _CACHE["runner"]

    import jax
    from jax.experimental.shard_map import shard_map
    from jax.sharding import Mesh, PartitionSpec

    nc = _build()
    bass2jax.install_neuronx_cc_hook()

    partition_name = (nc.partition_id_tensor.name
                      if nc.partition_id_tensor is not None else None)
    in_names = []
    out_names = []
    out_avals = []
    for alloc in nc.m.functions[0].allocations:
        if not isinstance(alloc, mybir.MemoryLocationSet):
            continue
        name = alloc.memorylocations[0].name
        if alloc.kind == "ExternalInput":
            if name != partition_name:
                in_names.append(name)
        elif alloc.kind == "ExternalOutput":
            out_names.append(name)
            out_avals.append(jax.core.ShapedArray(tuple(alloc.tensor_shape),
                                                  mybir.dt.np(alloc.dtype)))
    n_params = len(in_names)
    n_outs = len(out_names)
    all_in_names = in_names + out_names
    if partition_name is not None:
        all_in_names = all_in_names + [partition_name]

    def _body(*args):
        operands = list(args)
        if partition_name is not None:
            operands.append(bass2jax.partition_id_tensor())
        outs = bass2jax._bass_exec_p.bind(
            *operands,
            out_avals=tuple(out_avals),
            in_names=tuple(all_in_names),
            out_names=tuple(out_names),
            lowering_input_output_aliases=(),
            sim_require_finite=True,
            sim_require_nnan=True,
            nc=nc,
        )
        return tuple(outs)

    devices = jax.devices()[:N_CORES]
    mesh = Mesh(np.asarray(devices), ("core",))
    in_specs = (PartitionSpec("core"),) * (n_params + n_outs)
    out_specs = (PartitionSpec("core"),) * n_outs
    donate = tuple(range(n_params, n_params + n_outs))
    sharded = jax.jit(
        shard_map(_body, mesh=mesh, in_specs=in_specs, out_specs=out_specs,
                  check_rep=False),
        donate_argnums=donate, keep_unused=True)

    zero_out_shapes = [(N_CORES * a.shape[0], *a.shape[1:]) for a in out_avals]
    zero_out_dtypes = [a.dtype for a in out_avals]

    def run(in_maps):
        concat_in = [
            np.concatenate([np.asarray(m[name]) for m in in_maps], axis=0)
            for name in in_names
        ]
        zeros = [np.zeros(s, d) for s, d in zip(zero_out_shapes, zero_out_dtypes)]
        out_arrs = sharded(*concat_in, *zeros)
        return [
            {name: np.asarray(out_arrs[i]).reshape(N_CORES, *out_avals[i].shape)[c]
             for i, name in enumerate(out_names)}
            for c in range(N_CORES)
        ]

    _CACHE["runner"] = run
    return run


def kernel(x_velocity, y_velocity, p, x_faces, y_faces, params):
    x_velocity = np.asarray(x_velocity, np.float32)
    y_velocity = np.asarray(y_velocity, np.float32)
    p = np.asarray(p, np.float32)
    assert np.array_equal(np.asarray(x_faces), np.arange(0, N, 2, np.int32))
    assert np.array_equal(np.asarray(y_faces), np.arange(1, N, 2, np.int32))

    wm, bm = _pack_params(params)
    run = _get_runner()
    in_maps = [
        {"u": x_velocity[c], "v": y_velocity[c], "p": p[c],
         "wmega": wm, "bmega": bm}
        for c in range(N_CORES)
    ]
    res = run(in_maps)
    out_x = np.stack([res[c]["xedge"] for c in range(N_CORES)])
    out_y = np.stack([res[c]["yedge"] for c in range(N_CORES)])
    return (out_x, out_y)


# revision 13
# speedup vs baseline: 1.7627x; 1.2875x over previous
"""Trainium2 Bass kernel for nn_DeepConvectionNet.

Strategy (data-parallel over batch, one sample per NeuronCore, 8 cores):
  per core:
    1. compute the 5 patch fields (p_norm, r_x, r_y, u_norm, v_norm) on-chip
       from the raw sample, write them (plus raw u, v) as zero-padded
       [138, 262] planes to DRAM scratch.
    2. build a DRAM "REP" image REPD[k=(g,f,dx), j] = plane_f[j + g*262 + dx]
       (bf16): the L1 matmul of the 7x7x5 stencil MLP becomes 3 accumulating
       matmuls (K=105,105,35) whose rhs are strided slices of a streamed
       [105, FBLK] SBUF tile — no im2col materialisation.
    3. run the 6-layer MLP stack (tanh every layer) in "transposed"
       activation layout [features(part), cells(free)] with bf16 matmuls
       (fp32 PSUM accumulate) + ScalarE tanh. x/y chains are interleaved
       per layer; remainder-chunk tanhs of both chains are merged into
       single ACTIVATE ops via 32/64-aligned PSUM output bases.
    4. heads: elementwise multiply of the 10 MLP outputs with gathered
       velocity taps, reduced over the 5 taps by tiny K=10 matmuls.
    5. results staged to DRAM in face order, interleaved to cell order in
       an epilogue.
"""
import sys

for _p in ("/opt/trn_rl_repo", "/root/.axon_site/_ro/trn_rl_repo"):
    if _p not in sys.path:
        sys.path.append(_p)

import numpy as np

import bass_rust
import concourse.bass as bass
import concourse.bacc as bacc
import concourse.bass_isa as bass_isa
import concourse.tile as tile
from concourse import mybir
from concourse import bass2jax

f32 = mybir.dt.float32
bf16 = mybir.dt.bfloat16
MMDT = bf16
AF = mybir.ActivationFunctionType
ALU = mybir.AluOpType
AX = mybir.AxisListType

N_CORES = 8
B, NY, NX = 8, 128, 256
N = NY * NX
PLW = NX + 6          # 262
PLROWS = 138
PLANE = PLROWS * PLW  # 36156
NBLK = 16
BR = NY // NBLK       # 8 grid rows per block
FD = BR * NX // 2     # 1024 cells per chain per block
FBLK = 3664           # REP tile free length (>= 6*262+7*262+255+1)
FH = 2096             # huv tile free length (>= 7*262+255+1)
FREP = 35108          # REPD row length (134*262)
FH2 = 33536           # HUVD row length
P_ATM = 101325.0
EPS = 1e-8

D1, D2, D3, D4, D5, D6 = 214, 185, 156, 127, 68, 10
R1, R2, R3 = D1 - 128, D2 - 128, D3 - 128   # 86, 57, 28
L1_GROUPS = [(0, 105), (3, 105), (6, 35)]   # (base_dy, K)


def _layout():
    """Column layout of packed weight / bias images.
    WCOLS[name] = (col0, row0, row1, dout)."""
    wcols = {}
    c = 0

    def add(name, r0, r1, d):
        nonlocal c
        wcols[name] = (c, r0, r1, d)
        c += d

    for ch in ("x", "y"):
        for gi, (_, k) in enumerate(L1_GROUPS):
            add(f"w1{ch}{gi}a", 0, k, 128)
            add(f"w1{ch}{gi}b", 0, k, R1)
        add(f"w2{ch}ka_a", 0, 128, 128)
        add(f"w2{ch}kb_a", 0, R1, 128)
        add(f"w2{ch}ka_b", 0, 128, R2)
        add(f"w2{ch}kb_b", 0, R1, R2)
        add(f"w3{ch}ka_a", 0, 128, 128)
        add(f"w3{ch}ka_b", 0, 128, R3)
        add(f"w4{ch}ka", 0, 128, D4)
    # consumers of merged rem activations: y variants live at offset bases
    add("w3xkb_a", 0, R2, 128)
    add("w3xkb_b", 0, R2, R3)
    add("w3ykb_a", 64, 64 + R2, 128)
    add("w3ykb_b", 64, 64 + R2, R3)
    add("w4xkb", 0, R3, D4)
    add("w4ykb", 32, 32 + R3, D4)
    add("w5", 0, D4, D5)
    add("w6", 0, D5, D6)
    add("e0x", 0, D6, 1)
    add("e1x", 0, D6, 1)
    add("e0y", 32, 32 + D6, 1)
    add("e1y", 32, 32 + D6, 1)
    wc = c

    bcols = {}
    c = 0

    def addb(name):
        nonlocal c
        bcols[name] = c
        c += 1

    for ch in ("x", "y"):
        for nm in ("b1a", "b1b", "b2a", "b3a", "b4"):
            addb(nm + ch)
    for nm in ("b5", "b2bm", "b3bm", "b6m"):
        addb(nm)
    return wcols, wc, bcols, c


WCOLS, WC, BCOLS, BC = _layout()


def _permute_w1(w1):
    groups = []
    for base_dy, ng in ((0, 3), (3, 3), (6, 1)):
        g_rows = np.empty((35 * ng, w1.shape[1]), np.float32)
        for g in range(ng):
            for fch in range(5):
                for dx in range(7):
                    g_rows[g * 35 + fch * 7 + dx] = \
                        w1[((base_dy + g) * 7 + dx) * 5 + fch]
        groups.append(g_rows)
    return groups


def _pack_params(params):
    wm = np.zeros((128, WC), np.float32)
    bm = np.zeros((128, BC), np.float32)

    def put(name, block):
        c0, r0, r1, d = WCOLS[name]
        assert block.shape == (r1 - r0, d), (name, block.shape)
        wm[r0:r1, c0:c0 + d] = block

    loc = {"x": params["local_x"], "y": params["local_y"]}
    Wl = {ch: [np.asarray(w, np.float32) for w in loc[ch]["W"]] for ch in "xy"}
    bl = {ch: [np.asarray(b, np.float32) for b in loc[ch]["b"]] for ch in "xy"}
    Wg = [np.asarray(w, np.float32) for w in params["global"]["W"]]
    bg = [np.asarray(b, np.float32) for b in params["global"]["b"]]

    for ch in ("x", "y"):
        W = Wl[ch]
        g1 = _permute_w1(W[0])
        for gi in range(3):
            put(f"w1{ch}{gi}a", g1[gi][:, 0:128])
            put(f"w1{ch}{gi}b", g1[gi][:, 128:D1])
        put(f"w2{ch}ka_a", W[1][0:128, 0:128])
        put(f"w2{ch}kb_a", W[1][128:D1, 0:128])
        put(f"w2{ch}ka_b", W[1][0:128, 128:D2])
        put(f"w2{ch}kb_b", W[1][128:D1, 128:D2])
        put(f"w3{ch}ka_a", W[2][0:128, 0:128])
        put(f"w3{ch}ka_b", W[2][0:128, 128:D3])
        put(f"w4{ch}ka", W[3][0:128, :])
    put("w3xkb_a", Wl["x"][2][128:D2, 0:128])
    put("w3xkb_b", Wl["x"][2][128:D2, 128:D3])
    put("w3ykb_a", Wl["y"][2][128:D2, 0:128])
    put("w3ykb_b", Wl["y"][2][128:D2, 128:D3])
    put("w4xkb", Wl["x"][3][128:D3, :])
    put("w4ykb", Wl["y"][3][128:D3, :])
    put("w5", Wg[0])
    put("w6", Wg[1])
    e0 = np.zeros((D6, 1), np.float32)
    e0[0:5] = 1.0
    e1 = np.zeros((D6, 1), np.float32)
    e1[5:10] = 1.0
    put("e0x", e0)
    put("e1x", e1)
    put("e0y", e0)
    put("e1y", e1)

    for ch in ("x", "y"):
        bm[0:128, BCOLS["b1a" + ch]] = bl[ch][0][0:128]
        bm[0:R1, BCOLS["b1b" + ch]] = bl[ch][0][128:D1]
        bm[0:128, BCOLS["b2a" + ch]] = bl[ch][1][0:128]
        bm[0:128, BCOLS["b3a" + ch]] = bl[ch][2][0:128]
        bm[0:D4, BCOLS["b4" + ch]] = bl[ch][3]
    bm[0:D5, BCOLS["b5"]] = bg[0]
    bm[0:R2, BCOLS["b2bm"]] = bl["x"][1][128:D2]
    bm[64:64 + R2, BCOLS["b2bm"]] = bl["y"][1][128:D2]
    bm[0:R3, BCOLS["b3bm"]] = bl["x"][2][128:D3]
    bm[32:32 + R3, BCOLS["b3bm"]] = bl["y"][2][128:D3]
    bm[0:D6, BCOLS["b6m"]] = bg[1]
    bm[32:32 + D6, BCOLS["b6m"]] = bg[1]
    return wm, bm


def _ap(tile_ap, extra_off, dims):
    return bass_rust.AP(tile_ap.tensor, tile_ap.offset + extra_off, dims)


def _build():
    nc = bacc.Bacc("TRN2", target_bir_lowering=False, debug=False,
                   num_devices=N_CORES)
    u_in = nc.declare_dram_parameter("u", [N], f32, isOutput=False)
    v_in = nc.declare_dram_parameter("v", [N], f32, isOutput=False)
    p_in = nc.declare_dram_parameter("p", [N], f32, isOutput=False)
    wm_in = nc.declare_dram_parameter("wmega", [128, WC], f32, isOutput=False)
    bm_in = nc.declare_dram_parameter("bmega", [128, BC], f32, isOutput=False)
    xe_out = nc.declare_dram_parameter("xedge", [N], f32, isOutput=True)
    ye_out = nc.declare_dram_parameter("yedge", [N], f32, isOutput=True)
    planes = nc.dram_tensor("planes", [7 * PLANE], f32)
    repd = nc.dram_tensor("repd", [105 * FREP], MMDT)
    huvd = nc.dram_tensor("huvd", [20 * FH2], f32)
    edges = nc.dram_tensor("edges", [2, 2, N // 2], f32)

    with tile.TileContext(nc) as tc:
        with tc.tile_pool(name="const", bufs=1) as cpool, \
             tc.tile_pool(name="stage", bufs=4) as spool, \
             tc.tile_pool(name="repm", bufs=3) as rpool, \
             tc.tile_pool(name="huv", bufs=3) as hpool, \
             tc.tile_pool(name="acts", bufs=1) as apool, \
             tc.tile_pool(name="prod", bufs=2) as ppool, \
             tc.tile_pool(name="psum", bufs=4, space="PSUM") as qpool:

            wm = cpool.tile([128, WC], MMDT, tag="wm")
            nc.gpsimd.dma_start(wm[:], wm_in[:])
            bm = cpool.tile([128, BC], f32, tag="bm")
            nc.sync.dma_start(bm[:], bm_in[:])

            def wsl(name):
                c0, r0, r1, d = WCOLS[name]
                return wm[r0:r1, c0:c0 + d]

            def bsl(name, d0, dn):
                c0 = BCOLS[name]
                return bm[d0:dn, c0:c0 + 1]

            # ---------------- field computation ----------------
            with tc.tile_pool(name="fields", bufs=1) as fpool:
                u_t = fpool.tile([NY, NX], f32, tag="u")
                v_t = fpool.tile([NY, NX], f32, tag="v")
                p_t = fpool.tile([NY, NX], f32, tag="p")
                nc.sync.dma_start(u_t[:], u_in[:].rearrange("(a b) -> a b", b=NX))
                nc.sync.dma_start(v_t[:], v_in[:].rearrange("(a b) -> a b", b=NX))
                nc.sync.dma_start(p_t[:], p_in[:].rearrange("(a b) -> a b", b=NX))

                def padtile(tag):
                    t = fpool.tile([NY, PLW], f32, tag=tag)
                    nc.vector.memset(t[:], 0.0)
                    return t

                pn, rx, ry, un, vn, up, vp = (padtile(t) for t in
                                              ("pn", "rx", "ry", "un", "vn",
                                               "up", "vp"))
                nc.vector.tensor_scalar_add(pn[:, 3:3 + NX], p_t[:], -P_ATM)
                nc.vector.tensor_copy(up[:, 3:3 + NX], u_t[:])
                nc.vector.tensor_copy(vp[:, 3:3 + NX], v_t[:])

                for src, dst, tagp in ((u_t, un, "nu"), (v_t, vn, "nv")):
                    mx = fpool.tile([NY, 1], f32, tag=tagp + "mx")
                    nc.vector.tensor_reduce(mx[:], src[:], axis=AX.X, op=ALU.max)
                    am = fpool.tile([NY, 1], f32, tag=tagp + "am")
                    nc.gpsimd.partition_all_reduce(am[:], mx[:], NY,
                                                   bass_isa.ReduceOp.max)
                    nc.vector.tensor_scalar_add(am[:], am[:], EPS)
                    rcp = fpool.tile([NY, 1], f32, tag=tagp + "rc")
                    nc.vector.reciprocal(rcp[:], am[:])
                    nc.vector.tensor_scalar_mul(dst[:, 3:3 + NX], src[:],
                                                rcp[:, 0:1])

                # r_x from u
                dsh = fpool.tile([NY, NX], f32, tag="dsh")
                nc.vector.tensor_copy(dsh[:, 0:NX - 1], u_t[:, 1:NX])
                nc.vector.tensor_copy(dsh[:, NX - 1:NX], u_t[:, NX - 1:NX])
                ush = fpool.tile([NY, NX], f32, tag="ush")
                nc.vector.memset(ush[:, 0:1], 1.0)
                nc.vector.tensor_copy(ush[:, 1:NX], u_t[:, 0:NX - 1])
                num = fpool.tile([NY, NX], f32, tag="num")
                nc.vector.tensor_tensor(num[:], u_t[:], ush[:], op=ALU.subtract)
                den = fpool.tile([NY, NX], f32, tag="den")
                nc.vector.tensor_tensor(den[:], dsh[:], u_t[:], op=ALU.subtract)
                nc.vector.tensor_scalar_add(den[:], den[:], EPS)
                rcp2 = fpool.tile([NY, NX], f32, tag="rcp2")
                nc.vector.reciprocal(rcp2[:], den[:])
                rr = fpool.tile([NY, NX], f32, tag="rr")
                nc.vector.tensor_tensor(rr[:], num[:], rcp2[:], op=ALU.mult)
                nc.vector.tensor_scalar(rx[:, 3:3 + NX], rr[:], 0.0, 2.0,
                                        op0=ALU.max, op1=ALU.min)

                # r_y from v
                dsh2 = fpool.tile([NY, NX], f32, tag="dsh2")
                nc.vector.memset(dsh2[0:1, :], 0.0)
                nc.sync.dma_start(dsh2[1:NY, :], v_t[0:NY - 1, :])
                ush2 = fpool.tile([NY, NX], f32, tag="ush2")
                nc.vector.memset(ush2[:], 0.0)
                nc.sync.dma_start(ush2[0:NY - 1, :], v_t[1:NY, :])
                num2 = fpool.tile([NY, NX], f32, tag="num")
                nc.vector.tensor_tensor(num2[:], v_t[:], ush2[:], op=ALU.subtract)
                den2 = fpool.tile([NY, NX], f32, tag="den")
                nc.vector.tensor_tensor(den2[:], dsh2[:], v_t[:], op=ALU.subtract)
                nc.vector.tensor_scalar_add(den2[:], den2[:], EPS)
                rcp3 = fpool.tile([NY, NX], f32, tag="rcp2")
                nc.vector.reciprocal(rcp3[:], den2[:])
                rr2 = fpool.tile([NY, NX], f32, tag="rr")
                nc.vector.tensor_tensor(rr2[:], num2[:], rcp3[:], op=ALU.mult)
                nc.vector.tensor_scalar(ry[:, 3:3 + NX], rr2[:], 0.0, 2.0,
                                        op0=ALU.max, op1=ALU.min)

                zt = cpool.tile([7, PLW], f32, tag="zt")
                nc.vector.memset(zt[:], 0.0)
                for fi, ft in enumerate((pn, rx, ry, un, vn, up, vp)):
                    base = fi * PLANE
                    nc.sync.dma_start(
                        _ap(planes[:], base + 3 * PLW, [[PLW, NY], [1, PLW]]),
                        ft[:])
                    nc.sync.dma_start(
                        _ap(planes[:], base, [[PLW, 3], [1, PLW]]), zt[0:3, :])
                    nc.sync.dma_start(
                        _ap(planes[:], base + 131 * PLW, [[PLW, 7], [1, PLW]]),
                        zt[:])

            # ---------------- REPD / HUVD build ----------------
            JC = FREP // 4
            for g in range(3):
                for jc in range(4):
                    nc.gpsimd.dma_start(
                        _ap(repd[:], g * 35 * FREP + jc * JC,
                            [[FREP * 7, 5], [FREP, 7], [1, JC]]),
                        _ap(planes[:], g * PLW + jc * JC,
                            [[PLANE, 5], [1, 7], [1, JC]]))
            for ri, (pl, off, step) in enumerate((
                    (5, 3 * PLW + 1, 1), (6, 3 * PLW + 1, 1),
                    (5, PLW + 3, PLW), (6, PLW + 3, PLW))):
                nc.sync.dma_start(
                    _ap(huvd[:], ri * 5 * FH2, [[FH2, 5], [1, FH2]]),
                    _ap(planes[:], pl * PLANE + off, [[step, 5], [1, FH2]]))

            # ---------------- main loop ----------------
            ns = [slice(0, 512), slice(512, 1024)]
            for blk in range(NBLK):
                j0 = blk * BR * PLW
                repm = rpool.tile([105, FBLK], MMDT, tag="repm")
                rap = repm[:]
                rpitch = rap.ap[0][0]
                nc.sync.dma_start(
                    repm[:], _ap(repd[:], j0, [[FREP, 105], [1, FBLK]]))
                # huv: rows 0-9 = x (u,v), rows 32-41 = y (u,v)
                huv = hpool.tile([42, FH], f32, tag="huv")
                nc.sync.dma_start(
                    huv[0:10, :], _ap(huvd[:], j0, [[FH2, 10], [1, FH]]))
                nc.sync.dma_start(
                    huv[32:42, :], _ap(huvd[:], 10 * FH2 + j0,
                                       [[FH2, 10], [1, FH]]))

                def l1_mms(ch, xpar, which, ps, dd):
                    for n in range(2):
                        for gi, (bdy, K) in enumerate(L1_GROUPS):
                            rhs = _ap(rap, bdy * PLW + n * 4 * PLW + xpar,
                                      [[rpitch, K], [PLW, 4], [2, 128]])
                            nc.tensor.matmul(ps[0:dd, ns[n]],
                                             wsl(f"w1{ch}{gi}{which}"), rhs,
                                             start=(gi == 0), stop=(gi == 2))

                # ---- L1 ----
                ps1ax = qpool.tile([128, FD], f32, tag="ps")
                l1_mms("x", 0, "a", ps1ax, 128)
                ps1bx = qpool.tile([128, FD], f32, tag="ps")
                l1_mms("x", 0, "b", ps1bx, R1)
                t1ax = apool.tile([128, FD], MMDT, tag="t1a")
                nc.scalar.activation(t1ax[:], ps1ax[:], AF.Tanh,
                                     bias=bsl("b1ax", 0, 128))
                t1bx = apool.tile([R1, FD], MMDT, tag="t1b")
                nc.scalar.activation(t1bx[:], ps1bx[0:R1, :], AF.Tanh,
                                     bias=bsl("b1bx", 0, R1))
                ps1ay = qpool.tile([128, FD], f32, tag="ps")
                l1_mms("y", 1, "a", ps1ay, 128)
                ps1by = qpool.tile([128, FD], f32, tag="ps")
                l1_mms("y", 1, "b", ps1by, R1)
                t1ay = apool.tile([128, FD], MMDT, tag="t1c")
                nc.scalar.activation(t1ay[:], ps1ay[:], AF.Tanh,
                                     bias=bsl("b1ay", 0, 128))
                t1by = apool.tile([R1, FD], MMDT, tag="t1d")
                nc.scalar.activation(t1by[:], ps1by[0:R1, :], AF.Tanh,
                                     bias=bsl("b1by", 0, R1))
                t1 = {"x": (t1ax, t1bx), "y": (t1ay, t1by)}

                # ---- L2 ----
                def l2_mms(ch, ps, col_a, col_b, dd, base):
                    ka, kb = t1[ch]
                    for n in range(2):
                        nc.tensor.matmul(ps[base:base + dd, ns[n]], wsl(col_a),
                                         ka[:, ns[n]], start=True, stop=False,
                                         skip_group_check=True)
                        nc.tensor.matmul(ps[base:base + dd, ns[n]], wsl(col_b),
                                         kb[:, ns[n]], start=False, stop=True,
                                         skip_group_check=True)

                ps2ax = qpool.tile([128, FD], f32, tag="ps")
                l2_mms("x", ps2ax, "w2xka_a", "w2xkb_a", 128, 0)
                t2ax = apool.tile([128, FD], MMDT, tag="t2a")
                nc.scalar.activation(t2ax[:], ps2ax[:], AF.Tanh,
                                     bias=bsl("b2ax", 0, 128))
                ps2ay = qpool.tile([128, FD], f32, tag="ps")
                l2_mms("y", ps2ay, "w2yka_a", "w2ykb_a", 128, 0)
                t2ay = apool.tile([128, FD], MMDT, tag="t2c")
                nc.scalar.activation(t2ay[:], ps2ay[:], AF.Tanh,
                                     bias=bsl("b2ay", 0, 128))
                ps2b = qpool.tile([128, FD], f32, tag="ps")
                l2_mms("x", ps2b, "w2xka_b", "w2xkb_b", R2, 0)
                l2_mms("y", ps2b, "w2yka_b", "w2ykb_b", R2, 64)
                t2b = apool.tile([64 + R2, FD], MMDT, tag="t2b")
                nc.scalar.activation(t2b[:], ps2b[0:64 + R2, :], AF.Tanh,
                                     bias=bsl("b2bm", 0, 64 + R2))

                # ---- L3 ----  (rhs kb = merged t2b; y at base 64)
                def l3_mms(ch, ps, col_a, col_b, dd, base):
                    ka = t2ax if ch == "x" else t2ay
                    kboff = 0 if ch == "x" else 64
                    for n in range(2):
                        nc.tensor.matmul(ps[base:base + dd, ns[n]], wsl(col_a),
                                         ka[:, ns[n]], start=True, stop=False,
                                         skip_group_check=True)
                        nc.tensor.matmul(ps[base:base + dd, ns[n]], wsl(col_b),
                                         t2b[kboff:kboff + R2, ns[n]],
                                         start=False, stop=True,
                                         skip_group_check=True)

                ps3ax = qpool.tile([128, FD], f32, tag="ps")
                l3_mms("x", ps3ax, "w3xka_a", "w3xkb_a", 128, 0)
                t3ax = apool.tile([128, FD], MMDT, tag="t3a")
                nc.scalar.activation(t3ax[:], ps3ax[:], AF.Tanh,
                                     bias=bsl("b3ax", 0, 128))
                ps3ay = qpool.tile([128, FD], f32, tag="ps")
                l3_mms("y", ps3ay, "w3yka_a", "w3ykb_a", 128, 0)
                t3ay = apool.tile([128, FD], MMDT, tag="t3c")
                nc.scalar.activation(t3ay[:], ps3ay[:], AF.Tanh,
                                     bias=bsl("b3ay", 0, 128))
                ps3b = qpool.tile([128, FD], f32, tag="ps")
                l3_mms("x", ps3b, "w3xka_b", "w3xkb_b", R3, 0)
                l3_mms("y", ps3b, "w3yka_b", "w3ykb_b", R3, 32)
                t3b = apool.tile([32 + R3, FD], MMDT, tag="t3b")
                nc.scalar.activation(t3b[:], ps3b[0:32 + R3, :], AF.Tanh,
                                     bias=bsl("b3bm", 0, 32 + R3))

                # ---- L4 ----
                t4 = {}
                for ch in ("x", "y"):
                    ka = t3ax if ch == "x" else t3ay
                    kboff = 0 if ch == "x" else 32
                    ps4 = qpool.tile([128, FD], f32, tag="ps")
                    for n in range(2):
                        nc.tensor.matmul(ps4[0:D4, ns[n]], wsl(f"w4{ch}ka"),
                                         ka[:, ns[n]], start=True, stop=False)
                        nc.tensor.matmul(ps4[0:D4, ns[n]], wsl(f"w4{ch}kb"),
                                         t3b[kboff:kboff + R3, ns[n]],
                                         start=False, stop=True)
                    tt = apool.tile([D4, FD], MMDT, tag="t4" + ch)
                    nc.scalar.activation(tt[:], ps4[0:D4, :], AF.Tanh,
                                         bias=bsl("b4" + ch, 0, D4))
                    t4[ch] = tt

                # ---- L5 ----
                t5 = {}
                for ch in ("x", "y"):
                    ps5 = qpool.tile([128, FD], f32, tag="ps")
                    for n in range(2):
                        nc.tensor.matmul(ps5[0:D5, ns[n]], wsl("w5"),
                                         t4[ch][:, ns[n]], start=True, stop=True)
                    tt = apool.tile([D5, FD], MMDT, tag="t5" + ch)
                    nc.scalar.activation(tt[:], ps5[0:D5, :], AF.Tanh,
                                         bias=bsl("b5", 0, D5))
                    t5[ch] = tt

                # ---- L6 ----  (merged: x at [0:10], y at [32:42])
                ps6 = qpool.tile([128, FD], f32, tag="ps")
                for n in range(2):
                    nc.tensor.matmul(ps6[0:D6, ns[n]], wsl("w6"),
                                     t5["x"][:, ns[n]], start=True, stop=True,
                                     skip_group_check=True)
                for n in range(2):
                    nc.tensor.matmul(ps6[32:32 + D6, ns[n]], wsl("w6"),
                                     t5["y"][:, ns[n]], start=True, stop=True,
                                     skip_group_check=True)
                t6 = apool.tile([32 + D6, FD], MMDT, tag="t6")
                nc.scalar.activation(t6[:], ps6[0:32 + D6, :], AF.Tanh,
                                     bias=bsl("b6m", 0, 32 + D6))

                # ---- heads ----
                hap = huv[:]
                hpitch = hap.ap[0][0]
                prod = ppool.tile([32 + D6, FD], MMDT, tag="prod")
                hin_x = _ap(hap, 0, [[hpitch, D6], [PLW, BR], [2, 128]])
                nc.vector.tensor_tensor(prod[0:D6, :], t6[0:D6, :], hin_x,
                                        op=ALU.mult)
                hin_y = _ap(hap, 32 * hpitch + 1,
                            [[hpitch, D6], [PLW, BR], [2, 128]])
                nc.vector.tensor_tensor(prod[32:32 + D6, :], t6[32:32 + D6, :],
                                        hin_y, op=ALU.mult)
                for ci, ch in enumerate(("x", "y")):
                    poff = 0 if ch == "x" else 32
                    psh0 = qpool.tile([1, FD], f32, tag="ps")
                    psh1 = qpool.tile([1, FD], f32, tag="ps")
                    for n in range(2):
                        nc.tensor.matmul(psh0[0:1, ns[n]], wsl("e0" + ch),
                                         prod[poff:poff + D6, ns[n]],
                                         start=True, stop=True)
                        nc.tensor.matmul(psh1[0:1, ns[n]], wsl("e1" + ch),
                                         prod[poff:poff + D6, ns[n]],
                                         start=True, stop=True)
                    stx = spool.tile([1, FD], f32, tag="st")
                    sty = spool.tile([1, FD], f32, tag="st")
                    nc.vector.tensor_copy(stx[:], psh0[0:1, :])
                    nc.vector.tensor_copy(sty[:], psh1[0:1, :])
                    nc.sync.dma_start(edges[0, ci, blk * FD:(blk + 1) * FD],
                                      stx[:])
                    nc.sync.dma_start(edges[1, ci, blk * FD:(blk + 1) * FD],
                                      sty[:])

            # epilogue: interleave even/odd faces back into cell order
            with tc.tile_pool(name="epi", bufs=1) as epool:
                for oi, out_t in ((0, xe_out), (1, ye_out)):
                    ot = epool.tile([NY, NX], f32, tag=f"oint{oi}")
                    opitch = ot[:].ap[0][0]
                    for xpar in (0, 1):
                        half = epool.tile([NY, NX // 2], f32, tag=f"oh{oi}{xpar}")
                        nc.sync.dma_start(
                            half[:],
                            edges[oi, xpar, :].rearrange("(a b) -> a b",
                                                         b=NX // 2))
                        nc.vector.tensor_copy(
                            _ap(ot[:], xpar, [[opitch, NY], [2, NX // 2]]),
                            half[:])
                    nc.sync.dma_start(out_t[:].rearrange("(a b) -> a b", b=NX),
                                      ot[:])
    nc.compile()
    return nc


_CACHE = {}


def _get_runner():
    if "runner" in _CACHE:
        return _CACHE["runner"]

    import jax
    from jax.experimental.shard_map import shard_map
    from jax.sharding import Mesh, PartitionSpec

    nc = _build()
    bass2jax.install_neuronx_cc_hook()

    partition_name = (nc.partition_id_tensor.name
                      if nc.partition_id_tensor is not None else None)
    in_names = []
    out_names = []
    out_avals = []
    for alloc in nc.m.functions[0].allocations:
        if not isinstance(alloc, mybir.MemoryLocationSet):
            continue
        name = alloc.memorylocations[0].name
        if alloc.kind == "ExternalInput":
            if name != partition_name:
                in_names.append(name)
        elif alloc.kind == "ExternalOutput":
            out_names.append(name)
            out_avals.append(jax.core.ShapedArray(tuple(alloc.tensor_shape),
                                                  mybir.dt.np(alloc.dtype)))
    n_params = len(in_names)
    n_outs = len(out_names)
    all_in_names = in_names + out_names
    if partition_name is not None:
        all_in_names = all_in_names + [partition_name]

    def _body(*args):
        operands = list(args)
        if partition_name is not None:
            operands.append(bass2jax.partition_id_tensor())
        outs = bass2jax._bass_exec_p.bind(
            *operands,
            out_avals=tuple(out_avals),
            in_names=tuple(all_in_names),
            out_names=tuple(out_names),
            lowering_input_output_aliases=(),
            sim_require_finite=True,
            sim_require_nnan=True,
            nc=nc,
        )
        return tuple(outs)

    devices = jax.devices()[:N_CORES]
    mesh = Mesh(np.asarray(devices), ("core",))
    in_specs = (PartitionSpec("core"),) * (n_params + n_outs)
    out_specs = (PartitionSpec("core"),) * n_outs
    donate = tuple(range(n_params, n_params + n_outs))
    sharded = jax.jit(
        shard_map(_body, mesh=mesh, in_specs=in_specs, out_specs=out_specs,
                  check_rep=False),
        donate_argnums=donate, keep_unused=True)

    zero_out_shapes = [(N_CORES * a.shape[0], *a.shape[1:]) for a in out_avals]
    zero_out_dtypes = [a.dtype for a in out_avals]

    def run(in_maps):
        concat_in = [
            np.concatenate([np.asarray(m[name]) for m in in_maps], axis=0)
            for name in in_names
        ]
        zeros = [np.zeros(s, d) for s, d in zip(zero_out_shapes,
                                                zero_out_dtypes)]
        out_arrs = sharded(*concat_in, *zeros)
        return [
            {name: np.asarray(out_arrs[i]).reshape(N_CORES,
                                                   *out_avals[i].shape)[c]
             for i, name in enumerate(out_names)}
            for c in range(N_CORES)
        ]

    _CACHE["runner"] = run
    return run


def kernel(x_velocity, y_velocity, p, x_faces, y_faces, params):
    x_velocity = np.asarray(x_velocity, np.float32)
    y_velocity = np.asarray(y_velocity, np.float32)
    p = np.asarray(p, np.float32)
    assert np.array_equal(np.asarray(x_faces), np.arange(0, N, 2, np.int32))
    assert np.array_equal(np.asarray(y_faces), np.arange(1, N, 2, np.int32))

    wm, bm = _pack_params(params)
    run = _get_runner()
    in_maps = [
        {"u": x_velocity[c], "v": y_velocity[c], "p": p[c],
         "wmega": wm, "bmega": bm}
        for c in range(N_CORES)
    ]
    res = run(in_maps)
    out_x = np.stack([res[c]["xedge"] for c in range(N_CORES)])
    out_y = np.stack([res[c]["yedge"] for c in range(N_CORES)])
    return (out_x, out_y)


# revision 18
# speedup vs baseline: 1.8159x; 1.0302x over previous
"""Trainium2 Bass kernel for nn_DeepConvectionNet.

Strategy (data-parallel over batch, one sample per NeuronCore, 8 cores):
  per core:
    1. compute the 5 patch fields (p_norm, r_x, r_y, u_norm, v_norm) on-chip
       from the raw sample, write them (plus raw u, v) as zero-padded
       [138, 262] planes to DRAM scratch.
    2. build a DRAM "REP" image REPD[k=(g,f,dx), j] = plane_f[j + g*262 + dx]
       (bf16): the L1 matmul of the 7x7x5 stencil MLP becomes 3 accumulating
       matmuls (K=105,105,35) whose rhs are strided slices of a streamed
       [105, FBLK] SBUF tile — no im2col materialisation.
    3. run the 6-layer MLP stack (tanh every layer) in "transposed"
       activation layout [features(part), cells(free)] with bf16 matmuls
       (fp32 PSUM accumulate) + ScalarE tanh. x/y chains are interleaved
       per layer; remainder-chunk tanhs of both chains are merged into
       single ACTIVATE ops via 32/64-aligned PSUM output bases.
    4. heads: elementwise multiply of the 10 MLP outputs with gathered
       velocity taps, reduced over the 5 taps by tiny K=10 matmuls.
    5. results staged to DRAM in face order, interleaved to cell order in
       an epilogue.
"""
import sys

for _p in ("/opt/trn_rl_repo", "/root/.axon_site/_ro/trn_rl_repo"):
    if _p not in sys.path:
        sys.path.append(_p)

import numpy as np

import bass_rust
import concourse.bass as bass
import concourse.bacc as bacc
import concourse.bass_isa as bass_isa
import concourse.tile as tile
from concourse import mybir
from concourse import bass2jax

f32 = mybir.dt.float32
bf16 = mybir.dt.bfloat16
MMDT = bf16
AF = mybir.ActivationFunctionType
ALU = mybir.AluOpType
AX = mybir.AxisListType

N_CORES = 8
B, NY, NX = 8, 128, 256
N = NY * NX
PLW = NX + 6          # 262
PLROWS = 138
PLANE = PLROWS * PLW  # 36156
NBLK = 16
BR = NY // NBLK       # 8 grid rows per block
FD = BR * NX // 2     # 1024 cells per chain per block
W2 = 134              # parity-plane width (128 data + 2 left + 4 right pad)
PMROWS = 141
PMP = PMROWS * W2     # parity plane size (18894)
FBLK = 1872           # REP tile free length (>= 13*134+128)
FH = 1072             # huv tile free length (>= 7*134+128)
FREP = 17952          # REPD row length (>= 15*8*134 + FBLK)
FH2 = 17152           # HUVD row length
P_ATM = 101325.0
EPS = 1e-8

D1, D2, D3, D4, D5, D6 = 214, 185, 156, 127, 68, 10
R1, R2, R3 = D1 - 128, D2 - 128, D3 - 128   # 86, 57, 28
L1_GROUPS = [(0, 105), (3, 105), (6, 35)]   # (base_dy, K)


def _layout():
    """Column layout of packed weight / bias images.
    WCOLS[name] = (col0, row0, row1, dout)."""
    wcols = {}
    c = 0

    def add(name, r0, r1, d):
        nonlocal c
        wcols[name] = (c, r0, r1, d)
        c += d

    for ch in ("x", "y"):
        for gi, (_, k) in enumerate(L1_GROUPS):
            add(f"w1{ch}{gi}a", 0, k, 128)
            add(f"w1{ch}{gi}b", 0, k, R1)
        add(f"w2{ch}ka_a", 0, 128, 128)
        add(f"w2{ch}kb_a", 0, R1, 128)
        add(f"w2{ch}ka_b", 0, 128, R2)
        add(f"w2{ch}kb_b", 0, R1, R2)
        add(f"w3{ch}ka_a", 0, 128, 128)
        add(f"w3{ch}ka_b", 0, 128, R3)
        add(f"w4{ch}ka", 0, 128, D4)
    # consumers of merged rem activations: y variants live at offset bases
    add("w3xkb_a", 0, R2, 128)
    add("w3xkb_b", 0, R2, R3)
    add("w3ykb_a", 64, 64 + R2, 128)
    add("w3ykb_b", 64, 64 + R2, R3)
    add("w4xkb", 0, R3, D4)
    add("w4ykb", 32, 32 + R3, D4)
    add("w5", 0, D4, D5)
    add("w6", 0, D5, D6)
    add("e0x", 0, D6, 1)
    add("e1x", 0, D6, 1)
    add("e0y", 32, 32 + D6, 1)
    add("e1y", 32, 32 + D6, 1)
    wc = c

    bcols = {}
    c = 0

    def addb(name):
        nonlocal c
        bcols[name] = c
        c += 1

    for ch in ("x", "y"):
        for nm in ("b1a", "b1b", "b2a", "b3a", "b4"):
            addb(nm + ch)
    for nm in ("b5", "b2bm", "b3bm", "b6m"):
        addb(nm)
    return wcols, wc, bcols, c


WCOLS, WC, BCOLS, BC = _layout()


def _permute_w1(w1):
    # row order (g, dx, f)
    groups = []
    for base_dy, ng in ((0, 3), (3, 3), (6, 1)):
        g_rows = np.empty((35 * ng, w1.shape[1]), np.float32)
        for g in range(ng):
            for dx in range(7):
                for fch in range(5):
                    g_rows[g * 35 + dx * 5 + fch] = \
                        w1[((base_dy + g) * 7 + dx) * 5 + fch]
        groups.append(g_rows)
    return groups


def _pack_params(params):
    wm = np.zeros((128, WC), np.float32)
    bm = np.zeros((128, BC), np.float32)

    def put(name, block):
        c0, r0, r1, d = WCOLS[name]
        assert block.shape == (r1 - r0, d), (name, block.shape)
        wm[r0:r1, c0:c0 + d] = block

    loc = {"x": params["local_x"], "y": params["local_y"]}
    Wl = {ch: [np.asarray(w, np.float32) for w in loc[ch]["W"]] for ch in "xy"}
    bl = {ch: [np.asarray(b, np.float32) for b in loc[ch]["b"]] for ch in "xy"}
    Wg = [np.asarray(w, np.float32) for w in params["global"]["W"]]
    bg = [np.asarray(b, np.float32) for b in params["global"]["b"]]

    for ch in ("x", "y"):
        W = Wl[ch]
        g1 = _permute_w1(W[0])
        for gi in range(3):
            put(f"w1{ch}{gi}a", g1[gi][:, 0:128])
            put(f"w1{ch}{gi}b", g1[gi][:, 128:D1])
        put(f"w2{ch}ka_a", W[1][0:128, 0:128])
        put(f"w2{ch}kb_a", W[1][128:D1, 0:128])
        put(f"w2{ch}ka_b", W[1][0:128, 128:D2])
        put(f"w2{ch}kb_b", W[1][128:D1, 128:D2])
        put(f"w3{ch}ka_a", W[2][0:128, 0:128])
        put(f"w3{ch}ka_b", W[2][0:128, 128:D3])
        put(f"w4{ch}ka", W[3][0:128, :])
    put("w3xkb_a", Wl["x"][2][128:D2, 0:128])
    put("w3xkb_b", Wl["x"][2][128:D2, 128:D3])
    put("w3ykb_a", Wl["y"][2][128:D2, 0:128])
    put("w3ykb_b", Wl["y"][2][128:D2, 128:D3])
    put("w4xkb", Wl["x"][3][128:D3, :])
    put("w4ykb", Wl["y"][3][128:D3, :])
    put("w5", Wg[0])
    put("w6", Wg[1])
    e0 = np.zeros((D6, 1), np.float32)
    e0[0:5] = 1.0
    e1 = np.zeros((D6, 1), np.float32)
    e1[5:10] = 1.0
    put("e0x", e0)
    put("e1x", e1)
    put("e0y", e0)
    put("e1y", e1)

    for ch in ("x", "y"):
        bm[0:128, BCOLS["b1a" + ch]] = bl[ch][0][0:128]
        bm[0:R1, BCOLS["b1b" + ch]] = bl[ch][0][128:D1]
        bm[0:128, BCOLS["b2a" + ch]] = bl[ch][1][0:128]
        bm[0:128, BCOLS["b3a" + ch]] = bl[ch][2][0:128]
        bm[0:D4, BCOLS["b4" + ch]] = bl[ch][3]
    bm[0:D5, BCOLS["b5"]] = bg[0]
    bm[0:R2, BCOLS["b2bm"]] = bl["x"][1][128:D2]
    bm[64:64 + R2, BCOLS["b2bm"]] = bl["y"][1][128:D2]
    bm[0:R3, BCOLS["b3bm"]] = bl["x"][2][128:D3]
    bm[32:32 + R3, BCOLS["b3bm"]] = bl["y"][2][128:D3]
    bm[0:D6, BCOLS["b6m"]] = bg[1]
    bm[32:32 + D6, BCOLS["b6m"]] = bg[1]
    return wm, bm


def _ap(tile_ap, extra_off, dims):
    return bass_rust.AP(tile_ap.tensor, tile_ap.offset + extra_off, dims)


def _build():
    nc = bacc.Bacc("TRN2", target_bir_lowering=False, debug=False,
                   num_devices=N_CORES)
    u_in = nc.declare_dram_parameter("u", [N], f32, isOutput=False)
    v_in = nc.declare_dram_parameter("v", [N], f32, isOutput=False)
    p_in = nc.declare_dram_parameter("p", [N], f32, isOutput=False)
    wm_in = nc.declare_dram_parameter("wmega", [128, WC], f32, isOutput=False)
    bm_in = nc.declare_dram_parameter("bmega", [128, BC], f32, isOutput=False)
    xe_out = nc.declare_dram_parameter("xedge", [N], f32, isOutput=True)
    ye_out = nc.declare_dram_parameter("yedge", [N], f32, isOutput=True)
    planes = nc.dram_tensor("planes", [7 * 2 * PMP], f32)  # [f, par, PMP]
    repd = nc.dram_tensor("repd", [2 * 105 * FREP], MMDT)   # [par, k, FREP]
    huvd = nc.dram_tensor("huvd", [2 * 10 * FH2], f32)      # [par, row, FH2]
    edges = nc.dram_tensor("edges", [2, 2, N // 2], f32)

    with tile.TileContext(nc) as tc:
        with tc.tile_pool(name="const", bufs=1) as cpool, \
             tc.tile_pool(name="stage", bufs=4) as spool, \
             tc.tile_pool(name="repm", bufs=3) as rpool, \
             tc.tile_pool(name="huv", bufs=3) as hpool, \
             tc.tile_pool(name="acts", bufs=1) as apool, \
             tc.tile_pool(name="prod", bufs=2) as ppool, \
             tc.tile_pool(name="psum", bufs=4, space="PSUM") as qpool:

            wm = cpool.tile([128, WC], MMDT, tag="wm")
            nc.gpsimd.dma_start(wm[:], wm_in[:])
            bm = cpool.tile([128, BC], f32, tag="bm")
            nc.sync.dma_start(bm[:], bm_in[:])

            def wsl(name):
                c0, r0, r1, d = WCOLS[name]
                return wm[r0:r1, c0:c0 + d]

            def bsl(name, d0, dn):
                c0 = BCOLS[name]
                return bm[d0:dn, c0:c0 + 1]

            # ---------------- field computation ----------------
            with tc.tile_pool(name="fields", bufs=1) as fpool:
                u_t = fpool.tile([NY, NX], f32, tag="u")
                v_t = fpool.tile([NY, NX], f32, tag="v")
                p_t = fpool.tile([NY, NX], f32, tag="p")
                nc.sync.dma_start(u_t[:], u_in[:].rearrange("(a b) -> a b", b=NX))
                nc.sync.dma_start(v_t[:], v_in[:].rearrange("(a b) -> a b", b=NX))
                nc.sync.dma_start(p_t[:], p_in[:].rearrange("(a b) -> a b", b=NX))

                # full-width field values [NY, NX] (fp32)
                pn = fpool.tile([NY, NX], f32, tag="pn")
                nc.vector.tensor_scalar_add(pn[:], p_t[:], -P_ATM)
                un = fpool.tile([NY, NX], f32, tag="un")
                vn = fpool.tile([NY, NX], f32, tag="vn")
                for src, dst, tagp in ((u_t, un, "nu"), (v_t, vn, "nv")):
                    mx = fpool.tile([NY, 1], f32, tag=tagp + "mx")
                    nc.vector.tensor_reduce(mx[:], src[:], axis=AX.X, op=ALU.max)
                    am = fpool.tile([NY, 1], f32, tag=tagp + "am")
                    nc.gpsimd.partition_all_reduce(am[:], mx[:], NY,
                                                   bass_isa.ReduceOp.max)
                    nc.vector.tensor_scalar_add(am[:], am[:], EPS)
                    rcp = fpool.tile([NY, 1], f32, tag=tagp + "rc")
                    nc.vector.reciprocal(rcp[:], am[:])
                    nc.vector.tensor_scalar_mul(dst[:], src[:], rcp[:, 0:1])

                # r_x from u
                dsh = fpool.tile([NY, NX], f32, tag="dsh")
                nc.vector.tensor_copy(dsh[:, 0:NX - 1], u_t[:, 1:NX])
                nc.vector.tensor_copy(dsh[:, NX - 1:NX], u_t[:, NX - 1:NX])
                ush = fpool.tile([NY, NX], f32, tag="ush")
                nc.vector.memset(ush[:, 0:1], 1.0)
                nc.vector.tensor_copy(ush[:, 1:NX], u_t[:, 0:NX - 1])
                num = fpool.tile([NY, NX], f32, tag="num")
                nc.vector.tensor_tensor(num[:], u_t[:], ush[:], op=ALU.subtract)
                den = fpool.tile([NY, NX], f32, tag="den")
                nc.vector.tensor_tensor(den[:], dsh[:], u_t[:], op=ALU.subtract)
                nc.vector.tensor_scalar_add(den[:], den[:], EPS)
                rcp2 = fpool.tile([NY, NX], f32, tag="rcp2")
                nc.vector.reciprocal(rcp2[:], den[:])
                rxf = fpool.tile([NY, NX], f32, tag="rxf")
                nc.vector.tensor_tensor(rxf[:], num[:], rcp2[:], op=ALU.mult)
                nc.vector.tensor_scalar(rxf[:], rxf[:], 0.0, 2.0,
                                        op0=ALU.max, op1=ALU.min)

                # r_y from v
                dsh2 = fpool.tile([NY, NX], f32, tag="dsh2")
                nc.vector.memset(dsh2[0:1, :], 0.0)
                nc.sync.dma_start(dsh2[1:NY, :], v_t[0:NY - 1, :])
                ush2 = fpool.tile([NY, NX], f32, tag="ush2")
                nc.vector.memset(ush2[:], 0.0)
                nc.sync.dma_start(ush2[0:NY - 1, :], v_t[1:NY, :])
                num2 = fpool.tile([NY, NX], f32, tag="num")
                nc.vector.tensor_tensor(num2[:], v_t[:], ush2[:], op=ALU.subtract)
                den2 = fpool.tile([NY, NX], f32, tag="den")
                nc.vector.tensor_tensor(den2[:], dsh2[:], v_t[:], op=ALU.subtract)
                nc.vector.tensor_scalar_add(den2[:], den2[:], EPS)
                rcp3 = fpool.tile([NY, NX], f32, tag="rcp2")
                nc.vector.reciprocal(rcp3[:], den2[:])
                ryf = fpool.tile([NY, NX], f32, tag="ryf")
                nc.vector.tensor_tensor(ryf[:], num2[:], rcp3[:], op=ALU.mult)
                nc.vector.tensor_scalar(ryf[:], ryf[:], 0.0, 2.0,
                                        op0=ALU.max, op1=ALU.min)

                # parity-split into padded [NY, W2] tiles and write planes
                zt = cpool.tile([10, W2], f32, tag="zt")
                nc.vector.memset(zt[:], 0.0)
                for fi, ft in enumerate((pn, rxf, ryf, un, vn, u_t, v_t)):
                    for par in (0, 1):
                        pmt = fpool.tile([NY, W2], f32, tag=f"pm{par}")
                        nc.vector.memset(pmt[:], 0.0)
                        nc.vector.tensor_copy(
                            pmt[:, 2:2 + NX // 2],
                            _ap(ft[:], par, [[NX, NY], [2, NX // 2]]))
                        base = (fi * 2 + par) * PMP
                        nc.sync.dma_start(
                            _ap(planes[:], base + 3 * W2, [[W2, NY], [1, W2]]),
                            pmt[:])
                        nc.sync.dma_start(
                            _ap(planes[:], base, [[W2, 3], [1, W2]]), zt[0:3, :])
                        nc.sync.dma_start(
                            _ap(planes[:], base + 131 * W2, [[W2, 10], [1, W2]]),
                            zt[:])

            # ---------------- REPD / HUVD build ----------------
            # REPD[par][k=(g,dx,f), j2] = pm[f, pf][j2 + shift]
            for par in (0, 1):
                for g in range(3):
                    for dx in range(7):
                        q = par + dx - 3
                        pf = q & 1
                        off = (q - pf) // 2
                        shift = g * W2 + off + 2
                        k0 = g * 35 + dx * 5
                        nc.gpsimd.dma_start(
                            _ap(repd[:], (par * 105 + k0) * FREP,
                                [[FREP, 5], [1, FREP]]),
                            _ap(planes[:], pf * PMP + shift,
                                [[2 * PMP, 5], [1, FREP]]))
            # HUVD x-chain (par=0): column taps at center row (+3)
            for t in range(2):
                for sp in range(5):
                    q = 0 + (sp - 2)
                    pf = q & 1
                    off = (q - pf) // 2
                    nc.sync.dma_start(
                        _ap(huvd[:], (t * 5 + sp) * FH2, [[1, FH2]]),
                        _ap(planes[:],
                            ((5 + t) * 2 + pf) * PMP + 3 * W2 + off + 2,
                            [[1, FH2]]))
            # HUVD y-chain (par=1): row taps at fixed column parity 1
            for t in range(2):
                nc.sync.dma_start(
                    _ap(huvd[:], (10 + t * 5) * FH2, [[FH2, 5], [1, FH2]]),
                    _ap(planes[:], ((5 + t) * 2 + 1) * PMP + W2 + 2,
                        [[W2, 5], [1, FH2]]))

            # ---------------- main loop ----------------
            ns = [slice(0, 512), slice(512, 1024)]
            for blk in range(NBLK):
                j0 = blk * BR * W2
                repm = {}
                for par in (0, 1):
                    rt = rpool.tile([105, FBLK], MMDT, tag=f"repm{par}")
                    nc.sync.dma_start(
                        rt[:], _ap(repd[:], par * 105 * FREP + j0,
                                   [[FREP, 105], [1, FBLK]]))
                    repm[par] = rt
                # huv: rows 0-9 = x (u,v), rows 32-41 = y (u,v)
                huv = hpool.tile([42, FH], f32, tag="huv")
                nc.sync.dma_start(
                    huv[0:10, :], _ap(huvd[:], j0, [[FH2, 10], [1, FH]]))
                nc.sync.dma_start(
                    huv[32:42, :], _ap(huvd[:], 10 * FH2 + j0,
                                       [[FH2, 10], [1, FH]]))

                def l1_mms(ch, xpar, which, ps, dd):
                    rap = repm[xpar][:]
                    rpitch = rap.ap[0][0]
                    for n in range(2):
                        for gi, (bdy, K) in enumerate(L1_GROUPS):
                            rhs = _ap(rap, bdy * W2 + n * 4 * W2,
                                      [[rpitch, K], [W2, 4], [1, 128]])
                            nc.tensor.matmul(ps[0:dd, ns[n]],
                                             wsl(f"w1{ch}{gi}{which}"), rhs,
                                             start=(gi == 0), stop=(gi == 2))

                # ---- L1 ----
                ps1ax = qpool.tile([128, FD], f32, tag="ps")
                l1_mms("x", 0, "a", ps1ax, 128)
                ps1bx = qpool.tile([128, FD], f32, tag="ps")
                l1_mms("x", 0, "b", ps1bx, R1)
                t1ax = apool.tile([128, FD], MMDT, tag="t1a")
                nc.scalar.activation(t1ax[:], ps1ax[:], AF.Tanh,
                                     bias=bsl("b1ax", 0, 128))
                t1bx = apool.tile([R1, FD], MMDT, tag="t1b")
                nc.scalar.activation(t1bx[:], ps1bx[0:R1, :], AF.Tanh,
                                     bias=bsl("b1bx", 0, R1))
                ps1ay = qpool.tile([128, FD], f32, tag="ps")
                l1_mms("y", 1, "a", ps1ay, 128)
                ps1by = qpool.tile([128, FD], f32, tag="ps")
                l1_mms("y", 1, "b", ps1by, R1)
                t1ay = apool.tile([128, FD], MMDT, tag="t1c")
                nc.scalar.activation(t1ay[:], ps1ay[:], AF.Tanh,
                                     bias=bsl("b1ay", 0, 128))
                t1by = apool.tile([R1, FD], MMDT, tag="t1d")
                nc.scalar.activation(t1by[:], ps1by[0:R1, :], AF.Tanh,
                                     bias=bsl("b1by", 0, R1))
                t1 = {"x": (t1ax, t1bx), "y": (t1ay, t1by)}

                # ---- L2 ----
                def l2_mms(ch, ps, col_a, col_b, dd, base):
                    ka, kb = t1[ch]
                    for n in range(2):
                        nc.tensor.matmul(ps[base:base + dd, ns[n]], wsl(col_a),
                                         ka[:, ns[n]], start=True, stop=False,
                                         skip_group_check=True)
                        nc.tensor.matmul(ps[base:base + dd, ns[n]], wsl(col_b),
                                         kb[:, ns[n]], start=False, stop=True,
                                         skip_group_check=True)

                ps2ax = qpool.tile([128, FD], f32, tag="ps")
                l2_mms("x", ps2ax, "w2xka_a", "w2xkb_a", 128, 0)
                t2ax = apool.tile([128, FD], MMDT, tag="t2a")
                nc.scalar.activation(t2ax[:], ps2ax[:], AF.Tanh,
                                     bias=bsl("b2ax", 0, 128))
                ps2ay = qpool.tile([128, FD], f32, tag="ps")
                l2_mms("y", ps2ay, "w2yka_a", "w2ykb_a", 128, 0)
                t2ay = apool.tile([128, FD], MMDT, tag="t2c")
                nc.scalar.activation(t2ay[:], ps2ay[:], AF.Tanh,
                                     bias=bsl("b2ay", 0, 128))
                ps2b = qpool.tile([128, FD], f32, tag="ps")
                l2_mms("x", ps2b, "w2xka_b", "w2xkb_b", R2, 0)
                l2_mms("y", ps2b, "w2yka_b", "w2ykb_b", R2, 64)
                t2b = apool.tile([64 + R2, FD], MMDT, tag="t2b")
                nc.scalar.activation(t2b[:], ps2b[0:64 + R2, :], AF.Tanh,
                                     bias=bsl("b2bm", 0, 64 + R2))

                # ---- L3 ----  (rhs kb = merged t2b; y at base 64)
                def l3_mms(ch, ps, col_a, col_b, dd, base):
                    ka = t2ax if ch == "x" else t2ay
                    kboff = 0 if ch == "x" else 64
                    for n in range(2):
                        nc.tensor.matmul(ps[base:base + dd, ns[n]], wsl(col_a),
                                         ka[:, ns[n]], start=True, stop=False,
                                         skip_group_check=True)
                        nc.tensor.matmul(ps[base:base + dd, ns[n]], wsl(col_b),
                                         t2b[kboff:kboff + R2, ns[n]],
                                         start=False, stop=True,
                                         skip_group_check=True)

                ps3ax = qpool.tile([128, FD], f32, tag="ps")
                l3_mms("x", ps3ax, "w3xka_a", "w3xkb_a", 128, 0)
                t3ax = apool.tile([128, FD], MMDT, tag="t3a")
                nc.scalar.activation(t3ax[:], ps3ax[:], AF.Tanh,
                                     bias=bsl("b3ax", 0, 128))
                ps3ay = qpool.tile([128, FD], f32, tag="ps")
                l3_mms("y", ps3ay, "w3yka_a", "w3ykb_a", 128, 0)
                t3ay = apool.tile([128, FD], MMDT, tag="t3c")
                nc.scalar.activation(t3ay[:], ps3ay[:], AF.Tanh,
                                     bias=bsl("b3ay", 0, 128))
                ps3b = qpool.tile([128, FD], f32, tag="ps")
                l3_mms("x", ps3b, "w3xka_b", "w3xkb_b", R3, 0)
                l3_mms("y", ps3b, "w3yka_b", "w3ykb_b", R3, 32)
                t3b = apool.tile([32 + R3, FD], MMDT, tag="t3b")
                nc.scalar.activation(t3b[:], ps3b[0:32 + R3, :], AF.Tanh,
                                     bias=bsl("b3bm", 0, 32 + R3))

                # ---- L4 ----
                t4 = {}
                for ch in ("x", "y"):
                    ka = t3ax if ch == "x" else t3ay
                    kboff = 0 if ch == "x" else 32
                    ps4 = qpool.tile([128, FD], f32, tag="ps")
                    for n in range(2):
                        nc.tensor.matmul(ps4[0:D4, ns[n]], wsl(f"w4{ch}ka"),
                                         ka[:, ns[n]], start=True, stop=False)
                        nc.tensor.matmul(ps4[0:D4, ns[n]], wsl(f"w4{ch}kb"),
                                         t3b[kboff:kboff + R3, ns[n]],
                                         start=False, stop=True)
                    tt = apool.tile([D4, FD], MMDT, tag="t4" + ch)
                    nc.scalar.activation(tt[:], ps4[0:D4, :], AF.Tanh,
                                         bias=bsl("b4" + ch, 0, D4))
                    t4[ch] = tt

                # ---- L5 ----
                t5 = {}
                for ch in ("x", "y"):
                    ps5 = qpool.tile([128, FD], f32, tag="ps")
                    for n in range(2):
                        nc.tensor.matmul(ps5[0:D5, ns[n]], wsl("w5"),
                                         t4[ch][:, ns[n]], start=True, stop=True)
                    tt = apool.tile([D5, FD], MMDT, tag="t5" + ch)
                    nc.scalar.activation(tt[:], ps5[0:D5, :], AF.Tanh,
                                         bias=bsl("b5", 0, D5))
                    t5[ch] = tt

                # ---- L6 ----  (merged: x at [0:10], y at [32:42])
                ps6 = qpool.tile([128, FD], f32, tag="ps")
                for n in range(2):
                    nc.tensor.matmul(ps6[0:D6, ns[n]], wsl("w6"),
                                     t5["x"][:, ns[n]], start=True, stop=True,
                                     skip_group_check=True)
                for n in range(2):
                    nc.tensor.matmul(ps6[32:32 + D6, ns[n]], wsl("w6"),
                                     t5["y"][:, ns[n]], start=True, stop=True,
                                     skip_group_check=True)
                t6 = apool.tile([32 + D6, FD], MMDT, tag="t6")
                nc.scalar.activation(t6[:], ps6[0:32 + D6, :], AF.Tanh,
                                     bias=bsl("b6m", 0, 32 + D6))

                # ---- heads ----
                hap = huv[:]
                hpitch = hap.ap[0][0]
                prod = ppool.tile([32 + D6, FD], MMDT, tag="prod")
                hin_x = _ap(hap, 0, [[hpitch, D6], [W2, BR], [1, 128]])
                nc.vector.tensor_tensor(prod[0:D6, :], t6[0:D6, :], hin_x,
                                        op=ALU.mult)
                hin_y = _ap(hap, 32 * hpitch,
                            [[hpitch, D6], [W2, BR], [1, 128]])
                nc.vector.tensor_tensor(prod[32:32 + D6, :], t6[32:32 + D6, :],
                                        hin_y, op=ALU.mult)
                psh = qpool.tile([97, FD], f32, tag="ps")
                for n in range(2):
                    nc.tensor.matmul(psh[0:1, ns[n]], wsl("e0x"),
                                     prod[0:D6, ns[n]], start=True, stop=True,
                                     skip_group_check=True,
                                     tile_position=(0, 0))
                    nc.tensor.matmul(psh[32:33, ns[n]], wsl("e1x"),
                                     prod[0:D6, ns[n]], start=True, stop=True,
                                     skip_group_check=True,
                                     tile_position=(0, 32))
                    nc.tensor.matmul(psh[64:65, ns[n]], wsl("e0y"),
                                     prod[32:32 + D6, ns[n]], start=True,
                                     stop=True, skip_group_check=True,
                                     tile_position=(32, 64))
                    nc.tensor.matmul(psh[96:97, ns[n]], wsl("e1y"),
                                     prod[32:32 + D6, ns[n]], start=True,
                                     stop=True, skip_group_check=True,
                                     tile_position=(32, 96))
                for row, oi, ci in ((0, 0, 0), (32, 1, 0), (64, 0, 1),
                                    (96, 1, 1)):
                    st = spool.tile([1, FD], f32, tag="st")
                    nc.vector.tensor_copy(st[:], psh[row:row + 1, :])
                    nc.sync.dma_start(edges[oi, ci, blk * FD:(blk + 1) * FD],
                                      st[:])

            # epilogue: interleave even/odd faces back into cell order
            with tc.tile_pool(name="epi", bufs=1) as epool:
                for oi, out_t in ((0, xe_out), (1, ye_out)):
                    ot = epool.tile([NY, NX], f32, tag=f"oint{oi}")
                    opitch = ot[:].ap[0][0]
                    for xpar in (0, 1):
                        half = epool.tile([NY, NX // 2], f32, tag=f"oh{oi}{xpar}")
                        nc.sync.dma_start(
                            half[:],
                            edges[oi, xpar, :].rearrange("(a b) -> a b",
                                                         b=NX // 2))
                        nc.vector.tensor_copy(
                            _ap(ot[:], xpar, [[opitch, NY], [2, NX // 2]]),
                            half[:])
                    nc.sync.dma_start(out_t[:].rearrange("(a b) -> a b", b=NX),
                                      ot[:])
    nc.compile()
    return nc


_CACHE = {}


def _get_runner():
    if "runner" in _CACHE:
        return _CACHE["runner"]

    import jax
    from jax.experimental.shard_map import shard_map
    from jax.sharding import Mesh, PartitionSpec

    nc = _build()
    bass2jax.install_neuronx_cc_hook()

    partition_name = (nc.partition_id_tensor.name
                      if nc.partition_id_tensor is not None else None)
    in_names = []
    out_names = []
    out_avals = []
    for alloc in nc.m.functions[0].allocations:
        if not isinstance(alloc, mybir.MemoryLocationSet):
            continue
        name = alloc.memorylocations[0].name
        if alloc.kind == "ExternalInput":
            if name != partition_name:
                in_names.append(name)
        elif alloc.kind == "ExternalOutput":
            out_names.append(name)
            out_avals.append(jax.core.ShapedArray(tuple(alloc.tensor_shape),
                                                  mybir.dt.np(alloc.dtype)))
    n_params = len(in_names)
    n_outs = len(out_names)
    all_in_names = in_names + out_names
    if partition_name is not None:
        all_in_names = all_in_names + [partition_name]

    def _body(*args):
        operands = list(args)
        if partition_name is not None:
            operands.append(bass2jax.partition_id_tensor())
        outs = bass2jax._bass_exec_p.bind(
            *operands,
            out_avals=tuple(out_avals),
            in_names=tuple(all_in_names),
            out_names=tuple(out_names),
            lowering_input_output_aliases=(),
            sim_require_finite=True,
            sim_require_nnan=True,
            nc=nc,
        )
        return tuple(outs)

    devices = jax.devices()[:N_CORES]
    mesh = Mesh(np.asarray(devices), ("core",))
    in_specs = (PartitionSpec("core"),) * (n_params + n_outs)
    out_specs = (PartitionSpec("core"),) * n_outs
    donate = tuple(range(n_params, n_params + n_outs))
    sharded = jax.jit(
        shard_map(_body, mesh=mesh, in_specs=in_specs, out_specs=out_specs,
                  check_rep=False),
        donate_argnums=donate, keep_unused=True)

    zero_out_shapes = [(N_CORES * a.shape[0], *a.shape[1:]) for a in out_avals]
    zero_out_dtypes = [a.dtype for a in out_avals]

    def run(in_maps):
        concat_in = [
            np.concatenate([np.asarray(m[name]) for m in in_maps], axis=0)
            for name in in_names
        ]
        zeros = [np.zeros(s, d) for s, d in zip(zero_out_shapes,
                                                zero_out_dtypes)]
        out_arrs = sharded(*concat_in, *zeros)
        return [
            {name: np.asarray(out_arrs[i]).reshape(N_CORES,
                                                   *out_avals[i].shape)[c]
             for i, name in enumerate(out_names)}
            for c in range(N_CORES)
        ]

    _CACHE["runner"] = run
    return run


def kernel(x_velocity, y_velocity, p, x_faces, y_faces, params):
    x_velocity = np.asarray(x_velocity, np.float32)
    y_velocity = np.asarray(y_velocity, np.float32)
    p = np.asarray(p, np.float32)
    assert np.array_equal(np.asarray(x_faces), np.arange(0, N, 2, np.int32))
    assert np.array_equal(np.asarray(y_faces), np.arange(1, N, 2, np.int32))

    wm, bm = _pack_params(params)
    run = _get_runner()
    in_maps = [
        {"u": x_velocity[c], "v": y_velocity[c], "p": p[c],
         "wmega": wm, "bmega": bm}
        for c in range(N_CORES)
    ]
    res = run(in_maps)
    out_x = np.stack([res[c]["xedge"] for c in range(N_CORES)])
    out_y = np.stack([res[c]["yedge"] for c in range(N_CORES)])
    return (out_x, out_y)
